# revision 1
# baseline (speedup 1.0000x reference)
"""Attention2d Trainium2 Bass kernel.

Reference computation (per batch element b of 8, one NeuronCore each):
    hn  = GroupNorm32(x) * gn1_scale + gn1_bias
    qkv = w_qkv @ hn + b_qkv          (1x1 conv == matmul over channels)
    per head h (8 heads, ch=64): q,k,v from qkv (torch reshape convention:
        head h uses rows h*192+{0..64,64..128,128..192})
    wgt = softmax((q*s)^T (k*s)), s = ch**-0.25
    a   = v @ wgt^T
    out = GroupNorm32(w_proj @ a + b_proj) ... * gn2_scale + gn2_bias
    y   = x + out

Device strategy (data-parallel over batch, 1 core per batch element):
  - channel-chunked layouts [128, 4, N] (partition = channel % 128-chunk)
  - S^T computed as k^T q in [s, t] layout so the softmax denominator folds
    into the second attention matmul via an appended ones-column on v^T
  - softmax without max-subtraction (logits are ~N(0, 0.2); exp is safe)
  - all big matmuls run as float32r (full PE rate; fp32 is 4x slower)
  - GroupNorm group stats via tiny selection matmuls across partitions
  - emission is pair-interleaved (Q/K of a head pair, then its two heads'
    S^T/exp/A, then the pair's softmax denominators) so the ACT-bound
    attention pipeline starts as early as possible
"""

import numpy as np

NHEAD = 8
GROUPS = 32
EPS = 1e-5
B, C, H, W = 8, 512, 32, 32
N = H * W            # 1024 spatial positions
CH = C // NHEAD      # 64 channels per head
P = 128              # partitions
KC = C // P          # 4 channel chunks
NT = N // 512        # 2 column tiles of 512
SC = N // P          # 8 s-chunks

_CACHE = {}


def _build_nc(debug_taps=False):
    import concourse.tile as tile
    from concourse import mybir, bacc
    from concourse.hw_specs import get_activation_tables

    f32 = mybir.dt.float32
    f32r = mybir.dt.float32r
    AF = mybir.ActivationFunctionType
    OP = mybir.AluOpType

    nc = bacc.Bacc("TRN2", target_bir_lowering=False, num_devices=8)

    x_d = nc.dram_tensor("x", [P, KC, N], f32, kind="ExternalInput")
    wq_d = nc.dram_tensor("wq", [P, KC, KC, P], f32r, kind="ExternalInput")
    wk_d = nc.dram_tensor("wk", [P, KC, KC, P], f32r, kind="ExternalInput")
    wv_d = nc.dram_tensor("wv", [P, KC, C], f32r, kind="ExternalInput")
    wp_d = nc.dram_tensor("wp", [P, KC, C], f32r, kind="ExternalInput")
    bq_d = nc.dram_tensor("bq", [P, KC], f32, kind="ExternalInput")
    bk_d = nc.dram_tensor("bk", [P, KC], f32, kind="ExternalInput")
    bv_d = nc.dram_tensor("bv", [1, C], f32r, kind="ExternalInput")
    bp_d = nc.dram_tensor("bp", [P, KC], f32, kind="ExternalInput")
    g1s_d = nc.dram_tensor("g1s", [P, KC], f32, kind="ExternalInput")
    g1b_d = nc.dram_tensor("g1b", [P, KC], f32, kind="ExternalInput")
    g2s_d = nc.dram_tensor("g2s", [P, KC], f32, kind="ExternalInput")
    g2b_d = nc.dram_tensor("g2b", [P, KC], f32, kind="ExternalInput")
    sel_d = nc.dram_tensor("sel", [P, KC, GROUPS], f32, kind="ExternalInput")
    selt_d = nc.dram_tensor("selt", [GROUPS, KC, P], f32, kind="ExternalInput")
    eh_d = nc.dram_tensor("eh", [P, 2, P], f32r, kind="ExternalInput")
    out_d = nc.dram_tensor("out", [P, KC, N], f32, kind="ExternalOutput")
    if debug_taps:
        dbg = {
            "d_hn": nc.dram_tensor("d_hn", [P, KC, N], f32, kind="ExternalOutput"),
            "d_q": nc.dram_tensor("d_q", [P, KC, N], f32, kind="ExternalOutput"),
            "d_k": nc.dram_tensor("d_k", [P, KC, N], f32, kind="ExternalOutput"),
            "d_vt": nc.dram_tensor("d_vt", [P, SC, NHEAD, CH + 1], f32, kind="ExternalOutput"),
            "d_exps0": nc.dram_tensor("d_exps0", [P, SC, N], f32, kind="ExternalOutput"),
            "d_au": nc.dram_tensor("d_au", [P, KC, N], f32, kind="ExternalOutput"),
            "d_proj": nc.dram_tensor("d_proj", [P, KC, N], f32, kind="ExternalOutput"),
        }

    with tile.TileContext(nc) as tc:
        with (
            tc.tile_pool(name="big", bufs=1) as big,
            tc.tile_pool(name="wpool", bufs=3) as wpool,
            tc.tile_pool(name="qpool", bufs=2) as qpool,
            tc.tile_pool(name="kpool", bufs=4) as kpool,
            tc.tile_pool(name="vtp", bufs=1) as vtp,
            tc.tile_pool(name="expp", bufs=2) as expp,
            tc.tile_pool(name="tmpp", bufs=2) as tmpp,
            tc.tile_pool(name="stp", bufs=2, space="PSUM") as stp,
            tc.tile_pool(name="apool", bufs=2, space="PSUM") as apool,
            tc.tile_pool(name="qp", bufs=2, space="PSUM") as qp,
        ):
            # ---------- persistent SBUF tiles ----------
            x_sb = big.tile([P, KC, N], f32, tag="x_sb")
            hn = big.tile([P, KC, N], f32r, tag="hn")          # later reused as a_u
            vt = vtp.tile([P, SC, NHEAD, CH + 1], f32r, tag="vp")
            sums0 = big.tile([1, KC, N], f32r, tag="sums0")
            sums = big.tile([P, N], f32r, tag="sums")
            sel_sb = big.tile([P, KC, GROUPS], f32, tag="sel_sb")
            selt_sb = big.tile([GROUPS, KC, P], f32, tag="selt_sb")
            eh_sb = big.tile([P, 2, P], f32r, tag="eh_sb")
            bq_sb = big.tile([P, KC], f32, tag="bq_sb")
            bk_sb = big.tile([P, KC], f32, tag="bk_sb")
            bp_sb = big.tile([P, KC], f32, tag="bp_sb")
            bv_sb = big.tile([1, C], f32r, tag="bv_sb")
            g1s = big.tile([P, KC], f32, tag="g1s")
            g1b = big.tile([P, KC], f32, tag="g1b")
            g2s = big.tile([P, KC], f32, tag="g2s")
            g2b = big.tile([P, KC], f32, tag="g2b")
            ones1 = big.tile([1, P], f32r, tag="ones1")
            eps_sb = big.tile([GROUPS, 1], f32, tag="eps_sb")
            ab1 = big.tile([P, KC, 2], f32, tag="ab1")
            ab2 = big.tile([P, KC, 2], f32, tag="ab2")

            # ---------- input DMAs ----------
            for k in range(KC):
                for hlf in range(2):
                    nc.sync.dma_start(
                        x_sb[:, k, hlf * 512 : (hlf + 1) * 512],
                        x_d[:, k, hlf * 512 : (hlf + 1) * 512],
                    )
            for sb, d in (
                (sel_sb, sel_d), (selt_sb, selt_d), (g1s, g1s_d), (g1b, g1b_d),
                (bq_sb, bq_d), (bk_sb, bk_d), (bp_sb, bp_d), (bv_sb, bv_d),
                (g2s, g2s_d), (g2b, g2b_d), (eh_sb, eh_d),
            ):
                nc.sync.dma_start(sb[:], d[:])
            wq_sb = wpool.tile([P, KC, KC, P], f32r, tag="w")
            wk_sb = wpool.tile([P, KC, KC, P], f32r, tag="w")
            for m in range(KC):
                nc.sync.dma_start(wq_sb[:, m], wq_d[:, m])
                nc.sync.dma_start(wk_sb[:, m], wk_d[:, m])
            wv_sb = wpool.tile([P, KC, C], f32r, tag="w")
            nc.sync.dma_start(wv_sb[:], wv_d[:])

            # Preload the combined ln+exp ACT table set once, so the bacc
            # table-load pass doesn't thrash between natural_log and
            # exp_and_others at every Ln/Exp transition (~2.7us per switch).
            _set_names = list(get_activation_tables(nc.m.arch).keys())
            _tl = mybir.InstLoadActFuncSet(
                name=nc.get_next_instruction_name(),
                ins=[],
                outs=[],
                act_func_set_id=_set_names.index("natural_log_exp_and_others"),
            )
            _tl.engine = mybir.EngineType.Activation
            nc.scalar.add_instruction(_tl)

            nc.vector.memset(eps_sb[:], EPS)
            nc.vector.memset(sums[:].bitcast(f32), 1.0)
            onescol = big.tile([P, 1], f32, tag="onescol")
            nc.vector.memset(onescol[:], 1.0)
            nc.vector.tensor_copy(
                out=vt[:, :, :, CH : CH + 1],
                in_=onescol[:, :, None, None].to_broadcast((P, SC, NHEAD, 1)),
            )
            nc.vector.tensor_copy(
                out=ones1[:], in_=onescol[0:1, :].to_broadcast((1, P))
            )

            # ---------- GroupNorm helper ----------
            def group_norm_stats(src_tile, gs, gb, ab, uniq):
                """Per-channel (A, B) for y = src*A + B from 32-group stats."""
                stat2 = big.tile([P, KC, 2], f32, tag=f"stat2{uniq}")
                mvs = tmpp.tile([P, KC, 2], f32, tag="mvs")
                for k in range(KC):
                    resh = src_tile[:, k, :].rearrange("p (s f) -> p s f", f=512)
                    stats = tmpp.tile([P, 2, 6], f32, tag="bnstats")
                    for si in range(2):
                        nc.vector.bn_stats(out=stats[:, si, :], in_=resh[:, si, :])
                    nc.vector.bn_aggr(out=mvs[:, k, :], in_=stats[:])
                # stat2 col0 = mean, col1 = E[x^2] = var + mean^2 (batched)
                musq = tmpp.tile([P, KC], f32, tag="musq")
                nc.vector.tensor_tensor(musq[:], mvs[:, :, 0], mvs[:, :, 0], OP.mult)
                nc.vector.tensor_tensor(stat2[:, :, 1], mvs[:, :, 1], musq[:], OP.add)
                nc.vector.tensor_copy(out=stat2[:, :, 0], in_=mvs[:, :, 0])
                ps_g = qp.tile([P, 512], f32, tag="qp")
                for k in range(KC):
                    nc.tensor.matmul(
                        ps_g[0:GROUPS, 0:2], sel_sb[:, k, :], stat2[:, k, :],
                        start=(k == 0), stop=(k == KC - 1),
                    )
                # group mean / E2 -> rstd = exp(-0.5*ln(var+eps))
                gstat = big.tile([GROUPS, 2], f32, tag=f"gstat{uniq}")
                gms = tmpp.tile([GROUPS, 2], f32, tag="gms")
                nc.vector.tensor_copy(out=gms[:], in_=ps_g[0:GROUPS, 0:2])
                nc.vector.tensor_copy(out=gstat[:, 0:1], in_=gms[:, 0:1])
                gvar = tmpp.tile([GROUPS, 1], f32, tag="gvar")
                gmusq = tmpp.tile([GROUPS, 1], f32, tag="gmusq")
                nc.vector.tensor_tensor(gmusq[:], gms[:, 0:1], gms[:, 0:1], OP.mult)
                nc.vector.tensor_tensor(gvar[:], gms[:, 1:2], gmusq[:], OP.subtract)
                nc.scalar.activation(out=gvar[:], in_=gvar[:], func=AF.Ln, bias=eps_sb[:])
                nc.scalar.activation(out=gstat[:, 1:2], in_=gvar[:], func=AF.Exp, scale=-0.5)
                # broadcast (mean, rstd) back to channels, all chunks into one
                # psum, then batched A = scale*rstd, B = bias - mean*A
                ps_c = qp.tile([P, 512], f32, tag="qp")
                for k in range(KC):
                    nc.tensor.matmul(
                        ps_c[:, k * 2 : k * 2 + 2], selt_sb[:, k, :], gstat[:],
                        start=True, stop=True,
                    )
                cst = tmpp.tile([P, KC, 2], f32, tag="cst")
                nc.vector.tensor_copy(out=cst[:], in_=ps_c[:, 0 : 2 * KC])
                nc.vector.tensor_tensor(ab[:, :, 0], gs[:, :], cst[:, :, 1], OP.mult)
                ma = tmpp.tile([P, KC], f32, tag="ma")
                nc.vector.tensor_tensor(ma[:], cst[:, :, 0], ab[:, :, 0], OP.mult)
                nc.vector.tensor_tensor(ab[:, :, 1], gb[:, :], ma[:], OP.subtract)

            # ---------- GN1 -> hn ----------
            group_norm_stats(x_sb, g1s, g1b, ab1, "1")
            for k in range(KC):
                nc.vector.tensor_scalar(
                    hn[:, k, :], x_sb[:, k, :],
                    ab1[:, k, 0:1], ab1[:, k, 1:2], OP.mult, OP.add,
                )

            # ---------- phase helpers ----------
            def conv_out(dst_ap, w_sb, bias_sb, m):
                for t in range(NT):
                    ps = qp.tile([P, 512], f32, tag="qp")
                    for k in range(KC):
                        nc.tensor.matmul(
                            ps[:, :],
                            w_sb[:, m, k, :],
                            hn[:, k, t * 512 : (t + 1) * 512],
                            start=(k == 0), stop=(k == KC - 1),
                        )
                    nc.vector.tensor_scalar(
                        dst_ap[..., t * 512 : (t + 1) * 512], ps[:],
                        bias_sb[:, m : m + 1], None, OP.add,
                    )

            def v_tiles():
                # v^T tiles [s-part, head-major channel]; bv applied as an
                # extra K=1 contraction row (ones x bv) in the accumulation
                for nt in range(SC):
                    ps = qp.tile([P, 512], f32, tag="qp")
                    for k in range(KC):
                        nc.tensor.matmul(
                            ps[:, :],
                            hn[:, k, nt * P : (nt + 1) * P],
                            wv_sb[:, k, :],
                            start=(k == 0), stop=False,
                        )
                    nc.tensor.matmul(
                        ps[:, :],
                        ones1[:, 0:P],
                        bv_sb[:],
                        start=False, stop=True,
                    )
                    nc.vector.tensor_copy(
                        out=vt[:, nt, :, 0:CH],
                        in_=ps[:, :].rearrange("p (h c) -> p h c", h=NHEAD),
                    )

            a_u = big.tile([P, KC, N], f32r, tag="a_u")

            def head(h, qt, kt):
                p, e = h // 2, h % 2
                rows = slice(64 * e, 64 * e + 64)
                exps = expp.tile([P, SC, N], f32r, tag="exps")
                for sc in range(SC):
                    ps_st = stp.tile([P, N], f32, tag="stp")
                    for t in range(NT):
                        nc.tensor.matmul(
                            ps_st[:, t * 512 : (t + 1) * 512],
                            kt[rows, sc * P : (sc + 1) * P],
                            qt[rows, t * 512 : (t + 1) * 512],
                            start=True, stop=True,
                        )
                    nc.scalar.activation(out=exps[:, sc, :], in_=ps_st[:], func=AF.Exp)
                srow = 32 * p + e if p < 3 else 32 * e
                for t in range(NT):
                    ps_a = apool.tile([P, 512], f32, tag="apool")
                    for sc in range(SC):
                        nc.tensor.matmul(
                            ps_a[0 : CH + 1, :],
                            vt[:, sc, h, :],
                            exps[:, sc, t * 512 : (t + 1) * 512],
                            start=(sc == 0), stop=(sc == SC - 1),
                        )
                    tsl = slice(t * 512, (t + 1) * 512)
                    if e == 0 or p == 3:
                        # direct copy to a legal 32-aligned base
                        nc.vector.tensor_copy(
                            out=sums[srow : srow + 1, tsl], in_=ps_a[CH : CH + 1, :]
                        )
                    else:
                        nc.vector.tensor_copy(out=sums0[0:1, p, tsl], in_=ps_a[CH : CH + 1, :])
                    nc.vector.tensor_copy(out=a_u[rows, p, tsl], in_=ps_a[0:CH, :])
                if e == 1 and p < 3:
                    nc.sync.dma_start(sums[srow : srow + 1, :], sums0[0:1, p : p + 1, :])
                if debug_taps and h == 0:
                    nc.sync.dma_start(dbg["d_q"][:, 0, :], qt[:].bitcast(f32))
                    nc.sync.dma_start(dbg["d_exps0"][:], exps[:].bitcast(f32))

            # ---------- qkv, then attention ----------
            wp_sb = None

            def pair_recip(p):
                # per-pair softmax denominators: recip = exp(-ln(sums)).
                # Matmul operand bases must be 0/32/64, so pairs 0-2 sit at
                # partitions 32p; pair 3 uses rows 0 and 32 via eh region 1.
                if p < 3:
                    prow, reg = slice(32 * p, 32 * p + 2), 0
                else:
                    prow, reg = slice(0, 33), 1
                nc.scalar.activation(out=sums[prow, :], in_=sums[prow, :], func=AF.Ln)
                nc.scalar.activation(out=sums[prow, :], in_=sums[prow, :], func=AF.Exp, scale=-1.0)
                for t in range(NT):
                    ps_rb = apool.tile([P, 512], f32, tag="apool")
                    tsl = slice(t * 512, (t + 1) * 512)
                    nc.tensor.matmul(
                        ps_rb[:, :],
                        eh_sb[prow, reg, :],
                        sums[prow, tsl],
                        start=True, stop=True,
                    )
                    nc.vector.tensor_tensor(a_u[:, p, tsl], a_u[:, p, tsl], ps_rb[:], OP.mult)

            for p in range(KC):
                qt = qpool.tile([P, N], f32r, tag="qt")
                conv_out(qt[:, :], wq_sb, bq_sb, p)
                kt = kpool.tile([P, N], f32r, tag="kt")
                conv_out(kt[:, :], wk_sb, bk_sb, p)
                if p == 0:
                    v_tiles()
                    if debug_taps:
                        nc.sync.dma_start(dbg["d_hn"][:], hn[:].bitcast(f32))
                        nc.sync.dma_start(dbg["d_vt"][:], vt[:].bitcast(f32))
                if debug_taps:
                    nc.sync.dma_start(dbg["d_k"][:, p, :], kt[:].bitcast(f32))
                head(2 * p, qt, kt)
                head(2 * p + 1, qt, kt)
                if p == 0:
                    wp_sb = wpool.tile([P, KC, C], f32r, tag="w")
                    nc.sync.dma_start(wp_sb[:], wp_d[:])
                    nc.sync.dma_start(out_d[:], x_d[:])
                if p >= 1:
                    pair_recip(p - 1)
            pair_recip(3)

            if debug_taps:
                nc.sync.dma_start(dbg["d_au"][:], a_u[:].bitcast(f32))

            # ---------- proj ----------
            proj = vtp.tile([P, KC, N], f32r, tag="vp")  # vt is dead; same slot
            for m in range(KC):
                ps = stp.tile([P, N], f32, tag="stp")
                for t in range(NT):
                    for k in range(KC):
                        nc.tensor.matmul(
                            ps[:, t * 512 : (t + 1) * 512],
                            wp_sb[:, k, m * P : (m + 1) * P],
                            a_u[:, k, t * 512 : (t + 1) * 512],
                            start=(k == 0), stop=(k == KC - 1),
                        )
                nc.vector.tensor_scalar(
                    proj[:, m, :], ps[:], bp_sb[:, m : m + 1], None, OP.add,
                )

            if debug_taps:
                nc.sync.dma_start(dbg["d_proj"][:], proj[:].bitcast(f32))

            # ---------- GN2 + residual ----------
            group_norm_stats(proj, g2s, g2b, ab2, "2")
            for k in range(KC):
                nc.vector.tensor_scalar(
                    proj[:, k, :], proj[:, k, :],
                    ab2[:, k, 0:1], ab2[:, k, 1:2], OP.mult, OP.add,
                )
                nc.gpsimd.dma_start(
                    out_d[:, k, :], proj[:, k, :].bitcast(f32),
                    accum_op=OP.add,
                )

    nc.compile()
    return nc


def _host_prep(x, gn1_scale, gn1_bias, w_qkv, b_qkv, w_proj, b_proj, gn2_scale, gn2_bias):
    """Build per-core input maps (numpy only)."""
    f = np.float32
    x = np.asarray(x, f)
    w_qkv = np.asarray(w_qkv, f)
    b_qkv = np.asarray(b_qkv, f)
    w_proj = np.asarray(w_proj, f)
    b_proj = np.asarray(b_proj, f)
    gn1_scale = np.asarray(gn1_scale, f)
    gn1_bias = np.asarray(gn1_bias, f)
    gn2_scale = np.asarray(gn2_scale, f)
    gn2_bias = np.asarray(gn2_bias, f)

    def chunk_vec(v):  # [C] -> [P, KC]
        return np.ascontiguousarray(v.reshape(KC, P).T)

    def chunk_mat(wt):  # [C, M] -> [P, KC, M]
        return np.ascontiguousarray(wt.reshape(KC, P, -1).transpose(1, 0, 2))

    idx = np.arange(NHEAD)[:, None] * (3 * CH) + np.arange(CH)[None, :]
    q_idx, k_idx, v_idx = idx.ravel(), (idx + CH).ravel(), (idx + 2 * CH).ravel()

    s2 = float(CH) ** -0.5
    def mtile(w):  # [P, KC, C] -> [P, M, KC, P]
        return np.ascontiguousarray(
            w.reshape(P, KC, KC, P).transpose(0, 2, 1, 3)
        )
    wq = mtile(chunk_mat(w_qkv[q_idx].T * s2))
    wk = mtile(chunk_mat(w_qkv[k_idx].T))
    wv = chunk_mat(w_qkv[v_idx].T)
    wp = chunk_mat(w_proj.T)
    bq = chunk_vec(b_qkv[q_idx] * s2)
    bk = chunk_vec(b_qkv[k_idx])
    bv = np.ascontiguousarray(b_qkv[v_idx].reshape(1, C))
    bp = chunk_vec(b_proj)

    cidx = np.arange(C)
    sel = np.zeros((P, KC, GROUPS), f)
    sel[cidx % P, cidx // P, cidx // 16] = 1.0 / 16.0
    selt = np.zeros((GROUPS, KC, P), f)
    selt[cidx // 16, cidx // P, cidx % P] = 1.0
    eh = np.zeros((P, 2, P), f)
    for pp in range(3):
        for c in range(P):
            eh[32 * pp + c // CH, 0, c] = 1.0
    for c in range(P):
        eh[32 * (c // CH), 1, c] = 1.0

    shared = {
        "wq": wq, "wk": wk, "wv": wv, "wp": wp,
        "bq": bq, "bk": bk, "bv": bv, "bp": bp,
        "g1s": chunk_vec(gn1_scale), "g1b": chunk_vec(gn1_bias),
        "g2s": chunk_vec(gn2_scale), "g2b": chunk_vec(gn2_bias),
        "sel": sel, "selt": selt, "eh": eh,
    }
    in_maps = []
    for b in range(B):
        xb = np.ascontiguousarray(
            x[b].reshape(C, N).reshape(KC, P, N).transpose(1, 0, 2)
        )
        in_maps.append({"x": xb, **shared})
    return in_maps


def _assemble(results):
    out = np.empty((B, C, H, W), np.float32)
    for b in range(B):
        ob = np.asarray(results[b]["out"])  # [P, KC, N]
        out[b] = ob.transpose(1, 0, 2).reshape(C, N).reshape(C, H, W)
    return out


def get_nc():
    if "nc" not in _CACHE:
        _CACHE["nc"] = _build_nc()
    return _CACHE["nc"]


def kernel(x, gn1_scale, gn1_bias, w_qkv, b_qkv, w_proj, b_proj, gn2_scale, gn2_bias):
    from concourse.bass_utils import run_bass_kernel_spmd

    nc = get_nc()
    in_maps = _host_prep(
        x, gn1_scale, gn1_bias, w_qkv, b_qkv, w_proj, b_proj, gn2_scale, gn2_bias
    )
    res = run_bass_kernel_spmd(nc, in_maps, core_ids=list(range(B)))
    return _assemble(res.results)



# revision 3
# speedup vs baseline: 1.1251x; 1.1251x over previous
"""Attention2d Trainium2 Bass kernel (v2).

Reference computation (per batch element b of 8, one NeuronCore each):
    hn  = GroupNorm32(x) * gn1_scale + gn1_bias
    qkv = w_qkv @ hn + b_qkv          (1x1 conv == matmul over channels)
    per head h (8 heads, ch=64): wgt = softmax((q*s)^T (k*s)), s = ch**-0.25
    a   = v @ wgt^T
    out = GroupNorm32(w_proj @ a + b_proj)
    y   = x + out

Device strategy (data-parallel over batch, 1 core per batch element):
  - channel-chunked layouts [128, 4, N] (partition = channel % 128-chunk)
  - q,k,v convs in bf16 (full PE rate), q/k evicted directly to fp8e4
    with a +/- alpha prescale folded into the weights
  - S^T = k^T q as fp8 DoubleRow matmuls (half cost), ktile-1 zero-padded
    since the per-head contraction is only 64 channels
  - exp on ACT with scale = 1/alpha^2; exps stored bf16 (triple-buffered so
    the ACT pipeline never waits on the A-side consumers)
  - A computed TRANSPOSED: out[t, c] = sum_s exps[s, t] v[s, c]; the softmax
    denominator rides along as a ones-column of v^T and lands per-partition,
    so normalization is one DVE reciprocal + broadcast multiply
  - a^T transposed back to channel-major via PE identity-transposes (bf16)
  - proj in bf16, GroupNorm2 via tiny selection matmuls, residual added by a
    DMA accumulate against an out[] prefilled with x
"""

import numpy as np

NHEAD = 8
GROUPS = 32
EPS = 1e-5
B, C, H, W = 8, 512, 32, 32
N = H * W            # 1024 spatial positions
CH = C // NHEAD      # 64 channels per head
P = 128              # partitions
KC = C // P          # 4 channel chunks
NT = N // 512        # 2 column tiles of 512
SC = N // P          # 8 s-chunks
TC = N // P          # 8 t-chunks
ALPHA = 8.0          # fp8 prescale per side (folded into wq/wk, bq/bk)
EXPSCALE = 1.0 / (ALPHA * ALPHA)

_CACHE = {}


def _build_nc(debug_taps=False):
    import concourse.tile as tile
    from concourse import mybir, bacc
    from concourse.hw_specs import get_activation_tables

    f32 = mybir.dt.float32
    bf16 = mybir.dt.bfloat16
    f8 = mybir.dt.float8e4
    AF = mybir.ActivationFunctionType
    OP = mybir.AluOpType
    DR = mybir.MatmulPerfMode.DoubleRow

    nc = bacc.Bacc("TRN2", target_bir_lowering=False, num_devices=8)

    x_d = nc.dram_tensor("x", [P, KC, N], f32, kind="ExternalInput")
    wq_d = nc.dram_tensor("wq", [P, KC, KC, P], bf16, kind="ExternalInput")
    wk_d = nc.dram_tensor("wk", [P, KC, KC, P], bf16, kind="ExternalInput")
    wv_d = nc.dram_tensor("wv", [P, KC, C], bf16, kind="ExternalInput")
    wp_d = nc.dram_tensor("wp", [P, KC, C], bf16, kind="ExternalInput")
    bq_d = nc.dram_tensor("bq", [P, KC], f32, kind="ExternalInput")
    bk_d = nc.dram_tensor("bk", [P, KC], f32, kind="ExternalInput")
    bv_d = nc.dram_tensor("bv", [P, KC], f32, kind="ExternalInput")
    bp_d = nc.dram_tensor("bp", [P, KC], f32, kind="ExternalInput")
    g1s_d = nc.dram_tensor("g1s", [P, KC], f32, kind="ExternalInput")
    g1b_d = nc.dram_tensor("g1b", [P, KC], f32, kind="ExternalInput")
    g2s_d = nc.dram_tensor("g2s", [P, KC], f32, kind="ExternalInput")
    g2b_d = nc.dram_tensor("g2b", [P, KC], f32, kind="ExternalInput")
    sel_d = nc.dram_tensor("sel", [P, KC, GROUPS], f32, kind="ExternalInput")
    selt_d = nc.dram_tensor("selt", [GROUPS, KC, P], f32, kind="ExternalInput")
    id_d = nc.dram_tensor("ident", [P, P], bf16, kind="ExternalInput")
    out_d = nc.dram_tensor("out", [P, KC, N], f32, kind="ExternalOutput")
    if debug_taps:
        dbg = {
            "d_hn": nc.dram_tensor("d_hn", [P, KC, N], bf16, kind="ExternalOutput"),
            "d_q8": nc.dram_tensor("d_q8", [P, KC, NT, 2, 512], f8, kind="ExternalOutput"),
            "d_k8": nc.dram_tensor("d_k8", [P, KC, SC, 2, P], f8, kind="ExternalOutput"),
            "d_vt": nc.dram_tensor("d_vt", [P, SC, NHEAD, CH + 1], bf16, kind="ExternalOutput"),
            "d_e0": nc.dram_tensor("d_e0", [P, SC, N], bf16, kind="ExternalOutput"),
            "d_au": nc.dram_tensor("d_au", [P, KC, N], bf16, kind="ExternalOutput"),
            "d_pj": nc.dram_tensor("d_pj", [P, KC, N], bf16, kind="ExternalOutput"),
        }

    with tile.TileContext(nc) as tc:
        with (
            tc.tile_pool(name="big", bufs=1) as big,
            tc.tile_pool(name="expp", bufs=3) as expp,
            tc.tile_pool(name="atp", bufs=2) as atp,
            tc.tile_pool(name="tmpp", bufs=2) as tmpp,
            tc.tile_pool(name="ps", bufs=1, space="PSUM") as ps,
        ):
            # ---------- persistent SBUF tiles ----------
            x_sb = big.tile([P, KC, N], f32, tag="x_sb")
            hn = big.tile([P, KC, N], bf16, tag="hn")
            q8t = big.tile([P, KC, NT, 2, 512], f8, tag="q8t")
            k8t = big.tile([P, KC, SC, 2, P], f8, tag="k8t")
            vt = big.tile([P, SC, NHEAD, CH + 1], bf16, tag="vt")
            a_u = big.tile([P, KC, N], bf16, tag="a_u")
            proj = big.tile([P, KC, N], bf16, tag="proj")
            wq_sb = big.tile([P, KC, KC, P], bf16, tag="wq_sb")
            wk_sb = big.tile([P, KC, KC, P], bf16, tag="wk_sb")
            wv_sb = big.tile([P, KC, C], bf16, tag="wv_sb")
            wp_sb = big.tile([P, KC, C], bf16, tag="wp_sb")
            ident = big.tile([P, P], bf16, tag="ident")
            sel_sb = big.tile([P, KC, GROUPS], f32, tag="sel_sb")
            selt_sb = big.tile([GROUPS, KC, P], f32, tag="selt_sb")
            bq_sb = big.tile([P, KC], f32, tag="bq_sb")
            bk_sb = big.tile([P, KC], f32, tag="bk_sb")
            bv_sb = big.tile([P, KC], f32, tag="bv_sb")
            bp_sb = big.tile([P, KC], f32, tag="bp_sb")
            g1s = big.tile([P, KC], f32, tag="g1s")
            g1b = big.tile([P, KC], f32, tag="g1b")
            g2s = big.tile([P, KC], f32, tag="g2s")
            g2b = big.tile([P, KC], f32, tag="g2b")
            eps_sb = big.tile([GROUPS, 1], f32, tag="eps_sb")
            ab1 = big.tile([P, KC, 2], f32, tag="ab1")
            ab2 = big.tile([P, KC, 2], f32, tag="ab2")

            # ---------- prologue ----------
            # zero the ktile-1 halves of the fp8 DR operands (Pool is idle)
            nc.gpsimd.memset(q8t[:, :, :, 1, :], 0.0)
            nc.gpsimd.memset(k8t[:, :, :, 1, :], 0.0)

            for k in range(KC):
                for hlf in range(2):
                    nc.sync.dma_start(
                        x_sb[:, k, hlf * 512 : (hlf + 1) * 512],
                        x_d[:, k, hlf * 512 : (hlf + 1) * 512],
                    )
            for sb, d in (
                (sel_sb, sel_d), (selt_sb, selt_d), (g1s, g1s_d), (g1b, g1b_d),
                (bq_sb, bq_d), (bk_sb, bk_d), (bv_sb, bv_d), (bp_sb, bp_d),
                (g2s, g2s_d), (g2b, g2b_d), (ident, id_d),
                (wq_sb, wq_d), (wk_sb, wk_d), (wv_sb, wv_d), (wp_sb, wp_d),
            ):
                nc.sync.dma_start(sb[:], d[:])

            # Preload the combined ln+exp ACT table set once (avoids ~1.3us
            # table swaps between Ln and Exp at every GN <-> attention switch)
            _set_names = list(get_activation_tables(nc.m.arch).keys())
            _tl = mybir.InstLoadActFuncSet(
                name=nc.get_next_instruction_name(),
                ins=[],
                outs=[],
                act_func_set_id=_set_names.index("natural_log_exp_and_others"),
            )
            _tl.engine = mybir.EngineType.Activation
            nc.scalar.add_instruction(_tl)

            nc.vector.memset(eps_sb[:], EPS)
            nc.vector.memset(vt[:, :, :, CH : CH + 1], 1.0)

            # ---------- GroupNorm helper (from baseline) ----------
            def group_norm_stats(src_tile, gs, gb, ab, uniq):
                """Per-channel (A, B) for y = src*A + B from 32-group stats."""
                stat2 = big.tile([P, KC, 2], f32, tag=f"stat2{uniq}")
                mvs = tmpp.tile([P, KC, 2], f32, tag="mvs")
                for k in range(KC):
                    resh = src_tile[:, k, :].rearrange("p (s f) -> p s f", f=512)
                    stats = tmpp.tile([P, 2, 6], f32, tag="bnstats")
                    for si in range(2):
                        nc.vector.bn_stats(out=stats[:, si, :], in_=resh[:, si, :])
                    nc.vector.bn_aggr(out=mvs[:, k, :], in_=stats[:])
                musq = tmpp.tile([P, KC], f32, tag="musq")
                nc.vector.tensor_tensor(musq[:], mvs[:, :, 0], mvs[:, :, 0], OP.mult)
                nc.vector.tensor_tensor(stat2[:, :, 1], mvs[:, :, 1], musq[:], OP.add)
                nc.vector.tensor_copy(out=stat2[:, :, 0], in_=mvs[:, :, 0])
                ps_g = ps.tile([P, 512], f32, tag="c", bufs=2)
                for k in range(KC):
                    nc.tensor.matmul(
                        ps_g[0:GROUPS, 0:2], sel_sb[:, k, :], stat2[:, k, :],
                        start=(k == 0), stop=(k == KC - 1),
                    )
                gstat = big.tile([GROUPS, 2], f32, tag=f"gstat{uniq}")
                gms = tmpp.tile([GROUPS, 2], f32, tag="gms")
                nc.vector.tensor_copy(out=gms[:], in_=ps_g[0:GROUPS, 0:2])
                nc.vector.tensor_copy(out=gstat[:, 0:1], in_=gms[:, 0:1])
                gvar = tmpp.tile([GROUPS, 1], f32, tag="gvar")
                gmusq = tmpp.tile([GROUPS, 1], f32, tag="gmusq")
                nc.vector.tensor_tensor(gmusq[:], gms[:, 0:1], gms[:, 0:1], OP.mult)
                nc.vector.tensor_tensor(gvar[:], gms[:, 1:2], gmusq[:], OP.subtract)
                nc.scalar.activation(out=gvar[:], in_=gvar[:], func=AF.Ln, bias=eps_sb[:])
                nc.scalar.activation(out=gstat[:, 1:2], in_=gvar[:], func=AF.Exp, scale=-0.5)
                ps_c = ps.tile([P, 512], f32, tag="c", bufs=2)
                for k in range(KC):
                    nc.tensor.matmul(
                        ps_c[:, k * 2 : k * 2 + 2], selt_sb[:, k, :], gstat[:],
                        start=True, stop=True,
                    )
                cst = tmpp.tile([P, KC, 2], f32, tag="cst")
                nc.vector.tensor_copy(out=cst[:], in_=ps_c[:, 0 : 2 * KC])
                nc.vector.tensor_tensor(ab[:, :, 0], gs[:, :], cst[:, :, 1], OP.mult)
                ma = tmpp.tile([P, KC], f32, tag="ma")
                nc.vector.tensor_tensor(ma[:], cst[:, :, 0], ab[:, :, 0], OP.mult)
                nc.vector.tensor_tensor(ab[:, :, 1], gb[:, :], ma[:], OP.subtract)

            # ---------- GN1 -> hn (bf16) ----------
            group_norm_stats(x_sb, g1s, g1b, ab1, "1")
            for k in range(KC):
                nc.vector.tensor_scalar(
                    hn[:, k, :], x_sb[:, k, :],
                    ab1[:, k, 0:1], ab1[:, k, 1:2], OP.mult, OP.add,
                )

            # ---------- conv helpers ----------
            def qk_conv(m):
                # q: evict [P, 512] per t-tile into q8t[:, m, t, 0, :]
                for t in range(NT):
                    psq = ps.tile([P, 512], f32, tag="c", bufs=2)
                    for k in range(KC):
                        nc.tensor.matmul(
                            psq[:, :], wq_sb[:, m, k, :],
                            hn[:, k, t * 512 : (t + 1) * 512],
                            start=(k == 0), stop=(k == KC - 1),
                        )
                    nc.vector.tensor_scalar(
                        q8t[:, m, t, 0, :], psq[:], bq_sb[:, m : m + 1], None, OP.add,
                    )
                # k: evict into 4 s-chunk slots per t-tile
                for t in range(NT):
                    psk = ps.tile([P, 512], f32, tag="c", bufs=2)
                    for k in range(KC):
                        nc.tensor.matmul(
                            psk[:, :], wk_sb[:, m, k, :],
                            hn[:, k, t * 512 : (t + 1) * 512],
                            start=(k == 0), stop=(k == KC - 1),
                        )
                    nc.vector.tensor_scalar(
                        k8t[:, m, 4 * t : 4 * t + 4, 0, :],
                        psk[:].rearrange("p (a b) -> p a b", a=4),
                        bk_sb[:, m : m + 1], None, OP.add,
                    )

            def v_conv(nt):
                psv = ps.tile([P, 512], f32, tag="c", bufs=2)
                for k in range(KC):
                    nc.tensor.matmul(
                        psv[:, :], hn[:, k, nt * P : (nt + 1) * P], wv_sb[:, k, :],
                        start=(k == 0), stop=(k == KC - 1),
                    )
                nc.vector.tensor_copy(
                    out=vt[:, nt, :, 0:CH],
                    in_=psv[:].rearrange("p (h c) -> p h c", h=NHEAD),
                )

            # ---------- attention helpers ----------
            def s_head(h):
                """S^T (fp8 DoubleRow) + exp per s-chunk; returns exps buf."""
                p, e = h // 2, h % 2
                rows = slice(64 * e, 64 * e + 64)
                eb = expp.tile([P, SC, N], bf16, tag="exps")
                for sc in range(SC):
                    ps_st = ps.tile([P, N], f32, tag="s", bufs=2)
                    for t in range(NT):
                        nc.tensor.matmul(
                            ps_st[:, t * 512 : (t + 1) * 512],
                            k8t[rows, p, sc, :, :],
                            q8t[rows, p, t, :, :],
                            start=True, stop=True, perf_mode=DR,
                        )
                    nc.scalar.activation(
                        out=eb[:, sc, :], in_=ps_st[:], func=AF.Exp, scale=EXPSCALE,
                    )
                return eb

            def a_head(h, eb):
                """A^T: out[t, c] += exps[s, t] v[s, c]; normalize by the
                ones-column denominator (per-partition) on eviction."""
                aTt = atp.tile([P, TC, CH], bf16, tag="aT")
                rc = atp.tile([P, TC, 1], f32, tag="rc")
                for half in range(2):
                    ps_a = ps.tile([P, 4, CH + 1], f32, tag="a", bufs=1)
                    for j in range(4):
                        tcc = 4 * half + j
                        for sc in range(SC):
                            nc.tensor.matmul(
                                ps_a[:, j, :],
                                eb[:, sc, tcc * P : (tcc + 1) * P],
                                vt[:, sc, h, :],
                                start=(sc == 0), stop=(sc == SC - 1),
                            )
                    hsl = slice(4 * half, 4 * half + 4)
                    nc.vector.reciprocal(rc[:, hsl, :], ps_a[:, :, CH : CH + 1])
                    nc.vector.tensor_tensor(
                        aTt[:, hsl, :], ps_a[:, :, 0:CH],
                        rc[:, hsl, :].to_broadcast((P, 4, CH)), OP.mult,
                    )
                return aTt

            def t_head(h, aTt, ps_tt):
                e = h % 2
                for tcc in range(TC):
                    nc.tensor.transpose(
                        ps_tt[64 * e : 64 * e + 64, tcc, :], aTt[:, tcc, :], ident[:],
                    )

            def t_evict(p, ps_tt):
                nc.vector.tensor_scalar(
                    a_u[:, p, :].rearrange("p (a b) -> p a b", a=TC),
                    ps_tt[:], bv_sb[:, p : p + 1], None, OP.add,
                )

            # ---------- main attention pipeline ----------
            qk_conv(0)
            eb0 = s_head(0)
            for nt in range(SC):
                v_conv(nt)
            eb1 = s_head(1)
            qk_conv(1)
            nc.sync.dma_start(out_d[:], x_d[:])  # residual prefill
            eb2 = s_head(2)
            ebs = {0: eb0, 1: eb1, 2: eb2}
            ps_tts = {}

            def pair_tail(h):
                # A + transposes for head h, then evict the pair tile
                p, e = h // 2, h % 2
                aTt = a_head(h, ebs.pop(h))
                if e == 0:
                    ps_tts[p] = ps.tile([P, TC, P], bf16, tag="t", bufs=1, name="ps_tt")
                t_head(h, aTt, ps_tts[p])
                if e == 1:
                    t_evict(p, ps_tts.pop(p))

            pair_tail(0)
            ebs[3] = s_head(3)
            qk_conv(2)
            pair_tail(1)
            ebs[4] = s_head(4)
            pair_tail(2)
            ebs[5] = s_head(5)
            qk_conv(3)
            pair_tail(3)
            ebs[6] = s_head(6)
            pair_tail(4)
            ebs[7] = s_head(7)
            pair_tail(5)
            pair_tail(6)
            pair_tail(7)

            if debug_taps:
                nc.sync.dma_start(dbg["d_hn"][:], hn[:])
                nc.sync.dma_start(dbg["d_q8"][:], q8t[:])
                nc.sync.dma_start(dbg["d_k8"][:], k8t[:])
                nc.sync.dma_start(dbg["d_vt"][:], vt[:])
                nc.sync.dma_start(dbg["d_e0"][:], eb0[:])
                nc.sync.dma_start(dbg["d_au"][:], a_u[:])

            # ---------- proj (bf16) ----------
            for m in range(KC):
                psp = ps.tile([P, N], f32, tag="s", bufs=2)
                for t in range(NT):
                    for k in range(KC):
                        nc.tensor.matmul(
                            psp[:, t * 512 : (t + 1) * 512],
                            wp_sb[:, k, m * P : (m + 1) * P],
                            a_u[:, k, t * 512 : (t + 1) * 512],
                            start=(k == 0), stop=(k == KC - 1),
                        )
                nc.vector.tensor_scalar(
                    proj[:, m, :], psp[:], bp_sb[:, m : m + 1], None, OP.add,
                )

            if debug_taps:
                nc.sync.dma_start(dbg["d_pj"][:], proj[:])

            # ---------- GN2 + residual (via DMA accumulate) ----------
            group_norm_stats(proj, g2s, g2b, ab2, "2")
            for k in range(KC):
                nc.vector.tensor_scalar(
                    x_sb[:, k, :], proj[:, k, :],
                    ab2[:, k, 0:1], ab2[:, k, 1:2], OP.mult, OP.add,
                )
                nc.gpsimd.dma_start(
                    out_d[:, k, :], x_sb[:, k, :], accum_op=OP.add,
                )

    nc.compile()
    return nc


def _host_prep(x, gn1_scale, gn1_bias, w_qkv, b_qkv, w_proj, b_proj, gn2_scale, gn2_bias):
    """Build per-core input maps (numpy only)."""
    import ml_dtypes

    f = np.float32
    bf = ml_dtypes.bfloat16
    x = np.asarray(x, f)
    w_qkv = np.asarray(w_qkv, f)
    b_qkv = np.asarray(b_qkv, f)
    w_proj = np.asarray(w_proj, f)
    b_proj = np.asarray(b_proj, f)
    gn1_scale = np.asarray(gn1_scale, f)
    gn1_bias = np.asarray(gn1_bias, f)
    gn2_scale = np.asarray(gn2_scale, f)
    gn2_bias = np.asarray(gn2_bias, f)

    def chunk_vec(v):  # [C] -> [P, KC]
        return np.ascontiguousarray(v.reshape(KC, P).T)

    def chunk_mat(wt):  # [C, M] -> [P, KC, M]
        return np.ascontiguousarray(wt.reshape(KC, P, -1).transpose(1, 0, 2))

    def mtile(w):  # [P, KC, C] -> [P, M, KC, P]
        return np.ascontiguousarray(w.reshape(P, KC, KC, P).transpose(0, 2, 1, 3))

    idx = np.arange(NHEAD)[:, None] * (3 * CH) + np.arange(CH)[None, :]
    q_idx, k_idx, v_idx = idx.ravel(), (idx + CH).ravel(), (idx + 2 * CH).ravel()

    s4 = float(CH) ** -0.25
    wq = mtile(chunk_mat(w_qkv[q_idx].T * (s4 * ALPHA))).astype(bf)
    wk = mtile(chunk_mat(w_qkv[k_idx].T * (s4 * ALPHA))).astype(bf)
    wv = chunk_mat(w_qkv[v_idx].T).astype(bf)
    wp = chunk_mat(w_proj.T).astype(bf)
    bq = chunk_vec(b_qkv[q_idx] * (s4 * ALPHA))
    bk = chunk_vec(b_qkv[k_idx] * (s4 * ALPHA))
    bv = chunk_vec(b_qkv[v_idx])
    bp = chunk_vec(b_proj)

    cidx = np.arange(C)
    sel = np.zeros((P, KC, GROUPS), f)
    sel[cidx % P, cidx // P, cidx // 16] = 1.0 / 16.0
    selt = np.zeros((GROUPS, KC, P), f)
    selt[cidx // 16, cidx // P, cidx % P] = 1.0
    ident = np.eye(P, dtype=bf)

    shared = {
        "wq": wq, "wk": wk, "wv": wv, "wp": wp,
        "bq": bq, "bk": bk, "bv": bv, "bp": bp,
        "g1s": chunk_vec(gn1_scale), "g1b": chunk_vec(gn1_bias),
        "g2s": chunk_vec(gn2_scale), "g2b": chunk_vec(gn2_bias),
        "sel": sel, "selt": selt, "ident": ident,
    }
    in_maps = []
    for b in range(B):
        xb = np.ascontiguousarray(
            x[b].reshape(C, N).reshape(KC, P, N).transpose(1, 0, 2)
        )
        in_maps.append({"x": xb, **shared})
    return in_maps


def _assemble(results):
    out = np.empty((B, C, H, W), np.float32)
    for b in range(B):
        ob = np.asarray(results[b]["out"])  # [P, KC, N]
        out[b] = ob.transpose(1, 0, 2).reshape(C, N).reshape(C, H, W)
    return out


def get_nc():
    if "nc" not in _CACHE:
        _CACHE["nc"] = _build_nc()
    return _CACHE["nc"]


def kernel(x, gn1_scale, gn1_bias, w_qkv, b_qkv, w_proj, b_proj, gn2_scale, gn2_bias):
    from concourse.bass_utils import run_bass_kernel_spmd

    nc = get_nc()
    in_maps = _host_prep(
        x, gn1_scale, gn1_bias, w_qkv, b_qkv, w_proj, b_proj, gn2_scale, gn2_bias
    )
    res = run_bass_kernel_spmd(nc, in_maps, core_ids=list(range(B)))
    return _assemble(res.results)


# revision 73
# speedup vs baseline: 1.2847x; 1.1418x over previous
"""Attention2d Trainium2 Bass kernel (v2).

Reference computation (per batch element b of 8, one NeuronCore each):
    hn  = GroupNorm32(x) * gn1_scale + gn1_bias
    qkv = w_qkv @ hn + b_qkv          (1x1 conv == matmul over channels)
    per head h (8 heads, ch=64): wgt = softmax((q*s)^T (k*s)), s = ch**-0.25
    a   = v @ wgt^T
    out = GroupNorm32(w_proj @ a + b_proj)
    y   = x + out

Device strategy (data-parallel over batch, 1 core per batch element):
  - channel-chunked layouts [128, 4, N] (partition = channel % 128-chunk)
  - x and out in bf16 (halves the HBM head/tail); hn kept f32 so the v-path
    sees only one rounding; q,k,v convs in f32r (full PE rate at free>=256)
  - q,k evicted to bf16 with a symmetric alpha*ch^-0.25 prescale folded into
    the weights; S^T = k^T q in bf16; exp on ACT with scale 1/alpha^2
  - exps stored bf16, triple-buffered so ACT never waits on the A-side
  - A computed TRANSPOSED: out[t, c] = sum_s exps[s, t] v[s, c]; the softmax
    denominator rides along as a ones-column of v^T and lands per-partition,
    so normalization is one DVE broadcast-divide on eviction
  - a^T transposed back to channel-major via PE identity-transposes (bf16)
  - proj split: k=0..2 partial-summed under the last heads' exps (re-injected
    at the tail via identity matmuls into pre-opened psum groups), GroupNorm2
    stats read psum directly on DVE while ACT does the bias-add evictions,
    residual added by a bf16 DMA-accumulate against out[] prefilled with x
  - PE p-state warmup: junk matmuls keep the Tensor engine continuously busy
    through the prologue so real matmuls start at peak clock
"""

import numpy as np

NHEAD = 8
GROUPS = 32
EPS = 1e-5
B, C, H, W = 8, 512, 32, 32
N = H * W            # 1024 spatial positions
CH = C // NHEAD      # 64 channels per head
P = 128              # partitions
KC = C // P          # 4 channel chunks
NT = N // 512        # 2 column tiles of 512
SC = N // P          # 8 s-chunks
TC = N // P          # 8 t-chunks
ALPHA = 8.0          # fp8 prescale per side (folded into wq/wk, bq/bk)
EXPSCALE = 1.0 / (ALPHA * ALPHA)

_CACHE = {}


def _build_nc(debug_taps=False):
    import concourse.tile as tile
    from concourse import mybir, bacc
    from concourse.hw_specs import get_activation_tables

    f32 = mybir.dt.float32
    f32r = mybir.dt.float32r
    bf16 = mybir.dt.bfloat16
    AF = mybir.ActivationFunctionType
    OP = mybir.AluOpType
    DR = mybir.MatmulPerfMode.DoubleRow

    nc = bacc.Bacc("TRN2", target_bir_lowering=False, num_devices=8)

    x_d = nc.dram_tensor("x", [P, KC, N], bf16, kind="ExternalInput")
    wq_d = nc.dram_tensor("wq", [P, KC, KC, P], f32r, kind="ExternalInput")
    wk_d = nc.dram_tensor("wk", [P, KC, KC, P], f32r, kind="ExternalInput")
    wv_d = nc.dram_tensor("wv", [P, KC, C], f32r, kind="ExternalInput")
    wp_d = nc.dram_tensor("wp", [P, KC, C], bf16, kind="ExternalInput")
    # bq, bk, bv, bp, g1s, g1b, g2s, g2b packed as one tensor (one DMA —
    # HWDGE dispatch is ~625ns per DMA and the prologue is latency-critical)
    bias_d = nc.dram_tensor("biases", [P, 8, KC], f32, kind="ExternalInput")
    sel_d = nc.dram_tensor("sel", [P, KC, GROUPS], f32, kind="ExternalInput")
    selt_d = nc.dram_tensor("selt", [GROUPS, KC, P], f32, kind="ExternalInput")
    id_d = nc.dram_tensor("ident", [P, P], bf16, kind="ExternalInput")
    out_d = nc.dram_tensor("out", [P, KC, N], bf16, kind="ExternalOutput")
    if debug_taps:
        dbg = {
            "d_hn": nc.dram_tensor("d_hn", [P, KC, N], f32, kind="ExternalOutput"),
            "d_q8": nc.dram_tensor("d_q8", [P, KC, NT, 512], bf16, kind="ExternalOutput"),
            "d_k8": nc.dram_tensor("d_k8", [P, KC, SC, P], bf16, kind="ExternalOutput"),
            "d_vt": nc.dram_tensor("d_vt", [P, SC, NHEAD, CH + 1], bf16, kind="ExternalOutput"),
            "d_e0": nc.dram_tensor("d_e0", [P, SC, N], bf16, kind="ExternalOutput"),
            "d_au": nc.dram_tensor("d_au", [P, KC, N], bf16, kind="ExternalOutput"),
            "d_pj": nc.dram_tensor("d_pj", [P, KC, N], bf16, kind="ExternalOutput"),
        }

    with tile.TileContext(nc) as tc:
        with (
            tc.tile_pool(name="big", bufs=1) as big,
            tc.tile_pool(name="expp", bufs=3) as expp,
            tc.tile_pool(name="atp", bufs=2) as atp,
            tc.tile_pool(name="tmpp", bufs=2) as tmpp,
            tc.tile_pool(name="ps", bufs=1, space="PSUM") as ps,
        ):
            # ---------- persistent SBUF tiles ----------
            x_sb = big.tile([P, KC, N], bf16, tag="x_sb")
            hn = big.tile([P, KC, N], f32r, tag="hn")
            qbt = big.tile([P, KC, NT, 512], bf16, tag="qbt")
            kbt = big.tile([P, KC, SC, P], bf16, tag="kbt")
            vt = big.tile([P, SC, NHEAD, CH + 1], bf16, tag="vt")
            a_u = big.tile([P, KC, N], bf16, tag="a_u")
            proj = big.tile([P, KC, N], bf16, tag="proj")
            stg = big.tile([P, KC, N], bf16, tag="stg")
            wq_sb = big.tile([P, KC, KC, P], f32r, tag="wq_sb")
            wk_sb = big.tile([P, KC, KC, P], f32r, tag="wk_sb")
            wv_sb = big.tile([P, KC, C], f32r, tag="wv_sb")
            wp_sb = big.tile([P, KC, C], bf16, tag="wp_sb")
            ident = big.tile([P, P], bf16, tag="ident")
            sel_sb = big.tile([P, KC, GROUPS], f32, tag="sel_sb")
            selt_sb = big.tile([GROUPS, KC, P], f32, tag="selt_sb")
            bias_sb = big.tile([P, 8, KC], f32, tag="bias_sb")
            bq_sb = bias_sb[:, 0]
            bk_sb = bias_sb[:, 1]
            bv_sb = bias_sb[:, 2]
            bp_sb = bias_sb[:, 3]
            g1s = bias_sb[:, 4]
            g1b = bias_sb[:, 5]
            g2s = bias_sb[:, 6]
            g2b = bias_sb[:, 7]
            eps_sb = big.tile([GROUPS, 1], f32, tag="eps_sb")
            ab1 = big.tile([P, KC, 2], f32, tag="ab1")
            ab2 = big.tile([P, KC, 2], f32, tag="ab2")

            # ---------- prologue ----------
            # x DMAs small-first so bn_stats start early, coarser later so
            # HWDGE dispatch (625ns/DMA) doesn't serialize the prologue
            nc.sync.dma_start(x_sb[:, 0, 0:512], x_d[:, 0, 0:512])
            nc.sync.dma_start(x_sb[:, 0, 512:1024], x_d[:, 0, 512:1024])
            nc.sync.dma_start(x_sb[:, 1, :], x_d[:, 1, :])
            nc.sync.dma_start(x_sb[:, 2, :], x_d[:, 2, :])
            nc.sync.dma_start(x_sb[:, 3, 0:512], x_d[:, 3, 0:512])
            nc.sync.dma_start(x_sb[:, 3, 512:1024], x_d[:, 3, 512:1024])
            for sb, d in (
                (wq_sb, wq_d), (sel_sb, sel_d), (selt_sb, selt_d),
                (bias_sb, bias_d), (wk_sb, wk_d), (ident, id_d),
                (wv_sb, wv_d), (wp_sb, wp_d),
            ):
                nc.sync.dma_start(sb[:], d[:])

            # Preload the combined ln+exp ACT table set once (avoids ~1.3us
            # table swaps between Ln and Exp at every GN <-> attention switch)
            _set_names = list(get_activation_tables(nc.m.arch).keys())
            _tl = mybir.InstLoadActFuncSet(
                name=nc.get_next_instruction_name(),
                ins=[],
                outs=[],
                act_func_set_id=_set_names.index("natural_log_exp_and_others"),
            )
            _tl.engine = mybir.EngineType.Activation
            nc.scalar.add_instruction(_tl)

            nc.vector.memset(eps_sb[:], EPS)
            nc.vector.memset(vt[:, :, :, CH : CH + 1], 1.0)

            # PE p-state warmup: the Tensor engine needs ~3us of continuous
            # execution to reach peak clock. Junk matmuls fill the idle time
            # while x loads / GN1 stats run, so real matmuls start at speed.
            jtile = big.tile([P, P], bf16, tag="jtile")
            nc.vector.memset(jtile[:], 0.0)

            def warmup(n, tag="t"):
                psj = ps.tile([P, P], f32, tag=tag, bufs=(1 if tag == "t" else 2), name="psj")
                for _ in range(n):
                    nc.tensor.matmul(psj[:], jtile[:], jtile[:], start=True, stop=True)

            warmup(50)

            # ---------- GroupNorm helpers (from baseline) ----------
            def gn_chunk(src_ap, mvs, k):
                """mean/var of one channel chunk -> mvs[:, k, :]."""
                resh = src_ap.rearrange("p (s f) -> p s f", f=512)
                stats = tmpp.tile([P, 2, 6], f32, tag="bnstats")
                for si in range(2):
                    nc.vector.bn_stats(out=stats[:, si, :], in_=resh[:, si, :])
                nc.vector.bn_aggr(out=mvs[:, k, :], in_=stats[:])

            def gn_stat2(mvs, stat2, k, bias=None):
                """stat2[:, k] = (mean, E[x^2]) for chunk k; `bias` shifts the
                mean (stats taken pre-bias); variance is bias-invariant."""
                mean = mvs[:, k, 0:1]
                if bias is not None:
                    meant = tmpp.tile([P, 1], f32, tag="meant")
                    nc.vector.tensor_tensor(meant[:], mean, bias, OP.add)
                    mean = meant[:]
                musq = tmpp.tile([P, 1], f32, tag="musq")
                nc.vector.tensor_tensor(musq[:], mean, mean, OP.mult)
                nc.vector.tensor_tensor(stat2[:, k, 1:2], mvs[:, k, 1:2], musq[:], OP.add)
                nc.vector.tensor_copy(out=stat2[:, k, 0:1], in_=mean)

            def gn_finish(stat2, gs, gb, ab, uniq, psg_tag="c"):
                """Per-channel (A, B) for y = src*A + B from 32-group stats."""
                ps_g = ps.tile([P, 512], f32, tag=psg_tag, bufs=2, name="ps_g")
                for k in range(KC):
                    nc.tensor.matmul(
                        ps_g[0:GROUPS, 0:2], sel_sb[:, k, :], stat2[:, k, :],
                        start=(k == 0), stop=(k == KC - 1),
                    )
                gstat = big.tile([GROUPS, 2], f32, tag=f"gstat{uniq}")
                gms = tmpp.tile([GROUPS, 2], f32, tag="gms")
                nc.vector.tensor_copy(out=gms[:], in_=ps_g[0:GROUPS, 0:2])
                nc.vector.tensor_copy(out=gstat[:, 0:1], in_=gms[:, 0:1])
                gvar = tmpp.tile([GROUPS, 1], f32, tag="gvar")
                gmusq = tmpp.tile([GROUPS, 1], f32, tag="gmusq")
                nc.vector.tensor_tensor(gmusq[:], gms[:, 0:1], gms[:, 0:1], OP.mult)
                nc.vector.tensor_tensor(gvar[:], gms[:, 1:2], gmusq[:], OP.subtract)
                nc.scalar.activation(out=gvar[:], in_=gvar[:], func=AF.Ln, bias=eps_sb[:])
                nc.scalar.activation(out=gstat[:, 1:2], in_=gvar[:], func=AF.Exp, scale=-0.5)
                ps_c = ps.tile([P, 512], f32, tag="c", bufs=2)
                for k in range(KC):
                    nc.tensor.matmul(
                        ps_c[:, k * 2 : k * 2 + 2], selt_sb[:, k, :], gstat[:],
                        start=True, stop=True,
                    )
                cst = tmpp.tile([P, KC, 2], f32, tag="cst")
                nc.vector.tensor_copy(out=cst[:], in_=ps_c[:, 0 : 2 * KC])
                nc.vector.tensor_tensor(ab[:, :, 0], gs[:, :], cst[:, :, 1], OP.mult)
                ma = tmpp.tile([P, KC], f32, tag="ma")
                nc.vector.tensor_tensor(ma[:], cst[:, :, 0], ab[:, :, 0], OP.mult)
                nc.vector.tensor_tensor(ab[:, :, 1], gb[:, :], ma[:], OP.subtract)

            # ---------- GN1 -> hn (bf16) ----------
            mvs1 = tmpp.tile([P, KC, 2], f32, tag="mvs")
            stat21 = big.tile([P, KC, 2], f32, tag="stat21")
            for k in range(KC):
                gn_chunk(x_sb[:, k, :], mvs1, k)
                gn_stat2(mvs1, stat21, k)
            gn_finish(stat21, g1s, g1b, ab1, "1")
            warmup(20)
            # split by column half so the first q/k conv tiles can start as
            # soon as the t0 halves of all four chunks are normalized
            for t in range(NT):
                for k in range(KC):
                    tsl = slice(t * 512, (t + 1) * 512)
                    nc.vector.tensor_scalar(
                        hn[:, k, tsl], x_sb[:, k, tsl],
                        ab1[:, k, 0:1], ab1[:, k, 1:2], OP.mult, OP.add,
                    )

            # ---------- conv helpers ----------
            def qk_conv(m):
                # t-major (q t0, k t0, q t1, k t1) so head (m,0)'s first
                # S^T chunks have their operands as early as possible
                for t in range(NT):
                    psq = ps.tile([P, 512], f32, tag="c", bufs=2)
                    for k in range(KC):
                        nc.tensor.matmul(
                            psq[:, :], wq_sb[:, m, k, :],
                            hn[:, k, t * 512 : (t + 1) * 512],
                            start=(k == 0), stop=(k == KC - 1),
                        )
                    nc.vector.tensor_scalar(
                        qbt[:, m, t, :], psq[:], bq_sb[:, m : m + 1], None, OP.add,
                    )
                    psk = ps.tile([P, 512], f32, tag="c", bufs=2)
                    for k in range(KC):
                        nc.tensor.matmul(
                            psk[:, :], wk_sb[:, m, k, :],
                            hn[:, k, t * 512 : (t + 1) * 512],
                            start=(k == 0), stop=(k == KC - 1),
                        )
                    nc.vector.tensor_scalar(
                        kbt[:, m, 4 * t : 4 * t + 4, :],
                        psk[:].rearrange("p (a b) -> p a b", a=4),
                        bk_sb[:, m : m + 1], None, OP.add,
                    )

            def v_conv(nt):
                psv = ps.tile([P, 512], f32, tag="c", bufs=2)
                for k in range(KC):
                    nc.tensor.matmul(
                        psv[:, :], hn[:, k, nt * P : (nt + 1) * P], wv_sb[:, k, :],
                        start=(k == 0), stop=(k == KC - 1),
                    )
                nc.vector.tensor_copy(
                    out=vt[:, nt, :, 0:CH],
                    in_=psv[:].rearrange("p (h c) -> p h c", h=NHEAD),
                )

            # ---------- attention helpers ----------
            def s_head(h):
                """S^T (bf16) + exp per s-chunk; returns exps buf."""
                p, e = h // 2, h % 2
                rows = slice(64 * e, 64 * e + 64)
                eb = expp.tile([P, SC, N], bf16, tag="exps")
                for sc in range(SC):
                    ps_st = ps.tile([P, N], f32, tag="s", bufs=2)
                    for t in range(NT):
                        nc.tensor.matmul(
                            ps_st[:, t * 512 : (t + 1) * 512],
                            kbt[rows, p, sc, :],
                            qbt[rows, p, t, :],
                            start=True, stop=True,
                        )
                        if h == 0 and sc == 0:
                            # half-granular first exp: ACT starts as soon as
                            # the very first S tile lands
                            tsl = slice(t * 512, (t + 1) * 512)
                            nc.scalar.activation(
                                out=eb[:, sc, tsl], in_=ps_st[:, tsl],
                                func=AF.Exp, scale=EXPSCALE,
                            )
                    if not (h == 0 and sc == 0):
                        nc.scalar.activation(
                            out=eb[:, sc, :], in_=ps_st[:], func=AF.Exp, scale=EXPSCALE,
                        )
                return eb

            def a_head(h, eb, tag="a"):
                """A^T: out[t, c] += exps[s, t] v[s, c]; normalize by the
                ones-column denominator (per-partition) on eviction.
                With tag != "a" (two free slots) both halves' matmuls are
                emitted back-to-back so the evictions pipeline."""
                aTt = atp.tile([P, TC, CH], bf16, tag="aT")
                rc = atp.tile([P, TC, 1], f32, tag="rc")
                tiles = []
                for half in range(2):
                    ps_a = ps.tile([P, 4, CH + 1], f32, tag=tag, bufs=(1 if tag == "a" else 2), name="ps_a")
                    tiles.append(ps_a)
                    for j in range(4):
                        tcc = 4 * half + j
                        for sc in range(SC):
                            nc.tensor.matmul(
                                ps_a[:, j, :],
                                eb[:, sc, tcc * P : (tcc + 1) * P],
                                vt[:, sc, h, :],
                                start=(sc == 0), stop=(sc == SC - 1),
                            )
                    if tag == "a":
                        hsl = slice(4 * half, 4 * half + 4)
                        nc.vector.reciprocal(rc[:, hsl, :], ps_a[:, :, CH : CH + 1])
                        nc.vector.tensor_tensor(
                            aTt[:, hsl, :], ps_a[:, :, 0:CH],
                            rc[:, hsl, :].to_broadcast((P, 4, CH)), OP.mult,
                        )
                if tag != "a":
                    for half in range(2):
                        hsl = slice(4 * half, 4 * half + 4)
                        nc.vector.reciprocal(rc[:, hsl, :], tiles[half][:, :, CH : CH + 1])
                        nc.vector.tensor_tensor(
                            aTt[:, hsl, :], tiles[half][:, :, 0:CH],
                            rc[:, hsl, :].to_broadcast((P, 4, CH)), OP.mult,
                        )
                return aTt

            def t_head(h, aTt, ps_tt):
                e = h % 2
                for tcc in range(TC):
                    nc.tensor.transpose(
                        ps_tt[64 * e : 64 * e + 64, tcc, :], aTt[:, tcc, :], ident[:],
                    )

            def t_evict(p, ps_tt):
                # split in t-halves so consumers of the first half start early
                for half in range(2):
                    nc.vector.tensor_scalar(
                        a_u[:, p, half * 512 : (half + 1) * 512].rearrange(
                            "p (a b) -> p a b", a=TC // 2
                        ),
                        ps_tt[:, 4 * half : 4 * half + 4, :],
                        bv_sb[:, p : p + 1], None, OP.add,
                    )

            # ---------- main attention pipeline ----------
            qk_conv(0)
            eb0 = s_head(0)
            # v-convs interleaved with head 1's S chunks: total PE work in
            # the first two heads' exp window barely fits, so spread it
            p1, e1 = 0, 1
            rows1 = slice(64, 128)
            eb1 = expp.tile([P, SC, N], bf16, tag="exps", name="eb1")
            for sc in range(SC):
                ps_st = ps.tile([P, N], f32, tag="s", bufs=2, name="ps_st")
                for t in range(NT):
                    nc.tensor.matmul(
                        ps_st[:, t * 512 : (t + 1) * 512],
                        kbt[rows1, 0, sc, :], qbt[rows1, 0, t, :],
                        start=True, stop=True,
                    )
                nc.scalar.activation(
                    out=eb1[:, sc, :], in_=ps_st[:], func=AF.Exp, scale=EXPSCALE,
                )
                v_conv(sc)
            qk_conv(1)
            nc.sync.dma_start(out_d[:], x_d[:])  # residual prefill
            eb2 = s_head(2)
            ebs = {0: eb0, 1: eb1, 2: eb2}
            ps_tts = {}

            def pair_tail(h, tag="a"):
                # A + transposes for head h, then evict the pair tile
                p, e = h // 2, h % 2
                aTt = a_head(h, ebs.pop(h), tag)
                if e == 0:
                    ps_tts[p] = ps.tile([P, TC, P], bf16, tag="t", bufs=1, name="ps_tt")
                t_head(h, aTt, ps_tts[p])
                if e == 1:
                    t_evict(p, ps_tts.pop(p))

            pair_tail(0)
            ebs[3] = s_head(3)
            qk_conv(2)
            pair_tail(1)
            ebs[4] = s_head(4)
            pair_tail(2)
            ebs[5] = s_head(5)
            qk_conv(3)
            pair_tail(3)
            ebs[6] = s_head(6)
            pair_tail(4)
            ebs[7] = s_head(7)
            pair_tail(5)
            # proj partial sums over the finished chunks (pairs 0..2) run
            # under the last heads' exps (in the conv psum slots, which are
            # free by now); only the k=3 contraction is left for the tail
            partial = big.tile([P, KC, N], bf16, tag="partial")
            for m in range(KC):
                for t in range(NT):
                    tsl = slice(t * 512, (t + 1) * 512)
                    psw = ps.tile([P, 512], f32, tag="c", bufs=2)
                    for k in range(KC - 1):
                        nc.tensor.matmul(
                            psw[:, :], wp_sb[:, k, m * P : (m + 1) * P],
                            a_u[:, k, tsl],
                            start=(k == 0), stop=(k == KC - 2),
                        )
                    nc.vector.tensor_copy(out=partial[:, m, tsl], in_=psw[:])
            pair_tail(6)
            # Pre-open the proj psum groups with the `partial` identity-adds
            # while the last head's exps run; only the k=3 matmuls (gated on
            # pair 3's a_u) remain for the tail. Slot map: m0/m1 in the two
            # "s" slots (free as the last exps drain), m2 in "a"+"t", m3 in
            # the "c" slots (after head 7's A halves).
            tailps = []
            for m in range(2):
                psp = ps.tile([P, N], f32, tag="s", bufs=2, name="psp")
                tailps.append([psp[:, 0:512], psp[:, 512:1024]])
                for t in range(NT):
                    nc.tensor.matmul(
                        tailps[m][t], ident[:], partial[:, m, t * 512 : (t + 1) * 512],
                        start=True, stop=False,
                    )
            pair_tail(7, tag="c")
            ph2a = ps.tile([P, 512], f32, tag="a", bufs=1, name="ph2a")
            ph2b = ps.tile([P, 512], f32, tag="t", bufs=1, name="ph2b")
            ph3a = ps.tile([P, 512], f32, tag="c", bufs=2, name="ph3a")
            ph3b = ps.tile([P, 512], f32, tag="c", bufs=2, name="ph3b")
            tailps.append([ph2a[:, :], ph2b[:, :]])
            tailps.append([ph3a[:, :], ph3b[:, :]])
            for m in (2, 3):
                for t in range(NT):
                    nc.tensor.matmul(
                        tailps[m][t], ident[:], partial[:, m, t * 512 : (t + 1) * 512],
                        start=True, stop=False,
                    )

            if debug_taps:
                nc.sync.dma_start(dbg["d_hn"][:], hn[:])
                nc.sync.dma_start(dbg["d_q8"][:], qbt[:])
                nc.sync.dma_start(dbg["d_k8"][:], kbt[:])
                nc.sync.dma_start(dbg["d_vt"][:], vt[:])
                nc.sync.dma_start(dbg["d_e0"][:], eb0[:])
                nc.sync.dma_start(dbg["d_au"][:], a_u[:])

            # ---------- proj tail: k=3 matmuls + stats + ACT evictions ----------
            mvs2 = tmpp.tile([P, KC, 2], f32, tag="mvs")
            stat22 = big.tile([P, KC, 2], f32, tag="stat22")
            for m in range(KC):
                halves = tailps[m]
                statp = tmpp.tile([P, 2, 6], f32, tag="bnstats")
                for t in range(NT):
                    tsl = slice(t * 512, (t + 1) * 512)
                    nc.tensor.matmul(
                        halves[t], wp_sb[:, KC - 1, m * P : (m + 1) * P],
                        a_u[:, KC - 1, tsl], start=False, stop=True,
                    )
                    nc.vector.bn_stats(out=statp[:, t, :], in_=halves[t])
                nc.vector.bn_aggr(out=mvs2[:, m, :], in_=statp[:])
                gn_stat2(mvs2, stat22, m, bias=bp_sb[:, m : m + 1])
                # bias-add eviction on the (tail-idle) ACT engine so the DVE
                # stats chain isn't also carrying the evictions
                for t in range(NT):
                    nc.scalar.activation(
                        out=proj[:, m, t * 512 : (t + 1) * 512], in_=halves[t],
                        func=AF.Identity, bias=bp_sb[:, m : m + 1],
                    )

            if debug_taps:
                nc.sync.dma_start(dbg["d_pj"][:], proj[:])

            # ---------- GN2 + residual (via bf16 DMA accumulate) ----------
            gn_finish(stat22, g2s, g2b, ab2, "2", psg_tag="s")
            for k in range(KC):
                nc.vector.tensor_scalar(
                    stg[:, k, :], proj[:, k, :],
                    ab2[:, k, 0:1], ab2[:, k, 1:2], OP.mult, OP.add,
                )
                if k % 2 == 1:
                    nc.gpsimd.dma_start(
                        out_d[:, k - 1 : k + 1, :], stg[:, k - 1 : k + 1, :],
                        accum_op=OP.add,
                    )

    nc.compile()
    return nc


def _host_prep(x, gn1_scale, gn1_bias, w_qkv, b_qkv, w_proj, b_proj, gn2_scale, gn2_bias):
    """Build per-core input maps (numpy only)."""
    import ml_dtypes

    f = np.float32
    bf = ml_dtypes.bfloat16
    x = np.asarray(x, f)
    w_qkv = np.asarray(w_qkv, f)
    b_qkv = np.asarray(b_qkv, f)
    w_proj = np.asarray(w_proj, f)
    b_proj = np.asarray(b_proj, f)
    gn1_scale = np.asarray(gn1_scale, f)
    gn1_bias = np.asarray(gn1_bias, f)
    gn2_scale = np.asarray(gn2_scale, f)
    gn2_bias = np.asarray(gn2_bias, f)

    def chunk_vec(v):  # [C] -> [P, KC]
        return np.ascontiguousarray(v.reshape(KC, P).T)

    def chunk_mat(wt):  # [C, M] -> [P, KC, M]
        return np.ascontiguousarray(wt.reshape(KC, P, -1).transpose(1, 0, 2))

    def mtile(w):  # [P, KC, C] -> [P, M, KC, P]
        return np.ascontiguousarray(w.reshape(P, KC, KC, P).transpose(0, 2, 1, 3))

    idx = np.arange(NHEAD)[:, None] * (3 * CH) + np.arange(CH)[None, :]
    q_idx, k_idx, v_idx = idx.ravel(), (idx + CH).ravel(), (idx + 2 * CH).ravel()

    s4 = float(CH) ** -0.25
    wq = mtile(chunk_mat(w_qkv[q_idx].T * (s4 * ALPHA)))
    wk = mtile(chunk_mat(w_qkv[k_idx].T * (s4 * ALPHA)))
    wv = chunk_mat(w_qkv[v_idx].T)
    wp = chunk_mat(w_proj.T).astype(bf)
    biases = np.stack(
        [
            chunk_vec(b_qkv[q_idx] * (s4 * ALPHA)),
            chunk_vec(b_qkv[k_idx] * (s4 * ALPHA)),
            chunk_vec(b_qkv[v_idx]),
            chunk_vec(b_proj),
            chunk_vec(gn1_scale), chunk_vec(gn1_bias),
            chunk_vec(gn2_scale), chunk_vec(gn2_bias),
        ],
        axis=1,
    )

    cidx = np.arange(C)
    sel = np.zeros((P, KC, GROUPS), f)
    sel[cidx % P, cidx // P, cidx // 16] = 1.0 / 16.0
    selt = np.zeros((GROUPS, KC, P), f)
    selt[cidx // 16, cidx // P, cidx % P] = 1.0
    ident = np.eye(P, dtype=bf)

    shared = {
        "wq": wq, "wk": wk, "wv": wv, "wp": wp,
        "biases": np.ascontiguousarray(biases),
        "sel": sel, "selt": selt, "ident": ident,
    }
    in_maps = []
    for b in range(B):
        xb = np.ascontiguousarray(
            x[b].reshape(C, N).reshape(KC, P, N).transpose(1, 0, 2)
        ).astype(bf)
        in_maps.append({"x": xb, **shared})
    return in_maps


def _assemble(results):
    out = np.empty((B, C, H, W), np.float32)
    for b in range(B):
        ob = np.asarray(results[b]["out"]).astype(np.float32)  # [P, KC, N]
        out[b] = ob.transpose(1, 0, 2).reshape(C, N).reshape(C, H, W)
    return out


def get_nc():
    if "nc" not in _CACHE:
        _CACHE["nc"] = _build_nc()
    return _CACHE["nc"]


def kernel(x, gn1_scale, gn1_bias, w_qkv, b_qkv, w_proj, b_proj, gn2_scale, gn2_bias):
    from concourse.bass_utils import run_bass_kernel_spmd

    nc = get_nc()
    in_maps = _host_prep(
        x, gn1_scale, gn1_bias, w_qkv, b_qkv, w_proj, b_proj, gn2_scale, gn2_bias
    )
    res = run_bass_kernel_spmd(nc, in_maps, core_ids=list(range(B)))
    return _assemble(res.results)


# revision 79
# speedup vs baseline: 1.2975x; 1.0100x over previous
"""Attention2d Trainium2 Bass kernel (v2).

Reference computation (per batch element b of 8, one NeuronCore each):
    hn  = GroupNorm32(x) * gn1_scale + gn1_bias
    qkv = w_qkv @ hn + b_qkv          (1x1 conv == matmul over channels)
    per head h (8 heads, ch=64): wgt = softmax((q*s)^T (k*s)), s = ch**-0.25
    a   = v @ wgt^T
    out = GroupNorm32(w_proj @ a + b_proj)
    y   = x + out

Device strategy (data-parallel over batch, 1 core per batch element):
  - channel-chunked layouts [128, 4, N] (partition = channel % 128-chunk)
  - x and out in bf16 (halves the HBM head/tail); hn kept f32 so the v-path
    sees only one rounding; q,k,v convs in f32r (full PE rate at free>=256)
  - q,k evicted to bf16 with a symmetric alpha*ch^-0.25 prescale folded into
    the weights; S^T = k^T q in bf16; exp on ACT with scale 1/alpha^2
  - exps stored bf16, triple-buffered so ACT never waits on the A-side
  - A computed TRANSPOSED: out[t, c] = sum_s exps[s, t] v[s, c]; the softmax
    denominator rides along as a ones-column of v^T and lands per-partition,
    so normalization is one DVE broadcast-divide on eviction
  - a^T transposed back to channel-major via PE identity-transposes (bf16)
  - proj split: k=0..2 partial-summed under the last heads' exps (re-injected
    at the tail via identity matmuls into pre-opened psum groups), GroupNorm2
    stats read psum directly on DVE while ACT does the bias-add evictions,
    residual added by a bf16 DMA-accumulate against out[] prefilled with x
  - PE p-state warmup: junk matmuls keep the Tensor engine continuously busy
    through the prologue so real matmuls start at peak clock
"""

import numpy as np

NHEAD = 8
GROUPS = 32
EPS = 1e-5
B, C, H, W = 8, 512, 32, 32
N = H * W            # 1024 spatial positions
CH = C // NHEAD      # 64 channels per head
P = 128              # partitions
KC = C // P          # 4 channel chunks
NT = N // 512        # 2 column tiles of 512
SC = N // P          # 8 s-chunks
TC = N // P          # 8 t-chunks
ALPHA = 8.0          # fp8 prescale per side (folded into wq/wk, bq/bk)
EXPSCALE = 1.0 / (ALPHA * ALPHA)

_CACHE = {}


def _build_nc(debug_taps=False):
    import concourse.tile as tile
    from concourse import mybir, bacc
    from concourse.hw_specs import get_activation_tables

    f32 = mybir.dt.float32
    f32r = mybir.dt.float32r
    bf16 = mybir.dt.bfloat16
    AF = mybir.ActivationFunctionType
    OP = mybir.AluOpType
    DR = mybir.MatmulPerfMode.DoubleRow

    nc = bacc.Bacc("TRN2", target_bir_lowering=False, num_devices=8)

    x_d = nc.dram_tensor("x", [P, KC, N], bf16, kind="ExternalInput")
    wq_d = nc.dram_tensor("wq", [P, KC, KC, P], f32r, kind="ExternalInput")
    wk_d = nc.dram_tensor("wk", [P, KC, KC, P], f32r, kind="ExternalInput")
    wv_d = nc.dram_tensor("wv", [P, KC, C], f32r, kind="ExternalInput")
    wp_d = nc.dram_tensor("wp", [P, KC, C], bf16, kind="ExternalInput")
    # bq, bk, bv, bp, g1s, g1b, g2s, g2b packed as one tensor (one DMA —
    # HWDGE dispatch is ~625ns per DMA and the prologue is latency-critical)
    bias_d = nc.dram_tensor("biases", [P, 8, KC], f32, kind="ExternalInput")
    sel_d = nc.dram_tensor("sel", [P, KC, GROUPS], f32, kind="ExternalInput")
    selt_d = nc.dram_tensor("selt", [GROUPS, KC, P], f32, kind="ExternalInput")
    id_d = nc.dram_tensor("ident", [P, P], bf16, kind="ExternalInput")
    out_d = nc.dram_tensor("out", [P, KC, N], bf16, kind="ExternalOutput")
    if debug_taps:
        dbg = {
            "d_hn": nc.dram_tensor("d_hn", [P, KC, N], f32, kind="ExternalOutput"),
            "d_q8": nc.dram_tensor("d_q8", [P, KC, NT, 512], bf16, kind="ExternalOutput"),
            "d_k8": nc.dram_tensor("d_k8", [P, KC, SC, P], bf16, kind="ExternalOutput"),
            "d_vt": nc.dram_tensor("d_vt", [P, SC, NHEAD, CH + 1], bf16, kind="ExternalOutput"),
            "d_e0": nc.dram_tensor("d_e0", [P, SC, N], bf16, kind="ExternalOutput"),
            "d_au": nc.dram_tensor("d_au", [P, KC, N], bf16, kind="ExternalOutput"),
            "d_pj": nc.dram_tensor("d_pj", [P, KC, N], bf16, kind="ExternalOutput"),
        }

    with tile.TileContext(nc) as tc:
        with (
            tc.tile_pool(name="big", bufs=1) as big,
            tc.tile_pool(name="expp", bufs=3) as expp,
            tc.tile_pool(name="atp", bufs=2) as atp,
            tc.tile_pool(name="tmpp", bufs=2) as tmpp,
            tc.tile_pool(name="ps", bufs=1, space="PSUM") as ps,
        ):
            # ---------- persistent SBUF tiles ----------
            x_sb = big.tile([P, KC, N], bf16, tag="x_sb")
            hn = big.tile([P, KC, N], f32r, tag="hn")
            qbt = big.tile([P, KC, NT, 512], bf16, tag="qbt")
            kbt = big.tile([P, KC, SC, P], bf16, tag="kbt")
            vt = big.tile([P, SC, NHEAD, CH + 1], bf16, tag="vt")
            a_u = big.tile([P, KC, N], bf16, tag="a_u")
            proj = big.tile([P, KC, N], bf16, tag="proj")
            stg = big.tile([P, KC, N], bf16, tag="stg")
            wq_sb = big.tile([P, KC, KC, P], f32r, tag="wq_sb")
            wk_sb = big.tile([P, KC, KC, P], f32r, tag="wk_sb")
            wv_sb = big.tile([P, KC, C], f32r, tag="wv_sb")
            wp_sb = big.tile([P, KC, C], bf16, tag="wp_sb")
            ident = big.tile([P, P], bf16, tag="ident")
            sel_sb = big.tile([P, KC, GROUPS], f32, tag="sel_sb")
            selt_sb = big.tile([GROUPS, KC, P], f32, tag="selt_sb")
            bias_sb = big.tile([P, 8, KC], f32, tag="bias_sb")
            bq_sb = bias_sb[:, 0]
            bk_sb = bias_sb[:, 1]
            bv_sb = bias_sb[:, 2]
            bp_sb = bias_sb[:, 3]
            g1s = bias_sb[:, 4]
            g1b = bias_sb[:, 5]
            g2s = bias_sb[:, 6]
            g2b = bias_sb[:, 7]
            eps_sb = big.tile([GROUPS, 1], f32, tag="eps_sb")
            ab1 = big.tile([P, KC, 2], f32, tag="ab1")
            ab2 = big.tile([P, KC, 2], f32, tag="ab2")

            # ---------- prologue ----------
            # x DMAs small-first so bn_stats start early, coarser later so
            # HWDGE dispatch (625ns/DMA) doesn't serialize the prologue
            nc.sync.dma_start(x_sb[:, 0, 0:512], x_d[:, 0, 0:512])
            nc.sync.dma_start(x_sb[:, 0, 512:1024], x_d[:, 0, 512:1024])
            nc.sync.dma_start(x_sb[:, 1, :], x_d[:, 1, :])
            nc.sync.dma_start(x_sb[:, 2, :], x_d[:, 2, :])
            nc.sync.dma_start(x_sb[:, 3, 0:512], x_d[:, 3, 0:512])
            nc.sync.dma_start(x_sb[:, 3, 512:1024], x_d[:, 3, 512:1024])
            for sb, d in (
                (wq_sb, wq_d), (sel_sb, sel_d), (selt_sb, selt_d),
                (bias_sb, bias_d), (wk_sb, wk_d), (ident, id_d),
                (wv_sb, wv_d), (wp_sb, wp_d),
            ):
                nc.sync.dma_start(sb[:], d[:])

            # Preload the combined ln+exp ACT table set once (avoids ~1.3us
            # table swaps between Ln and Exp at every GN <-> attention switch)
            _set_names = list(get_activation_tables(nc.m.arch).keys())
            _tl = mybir.InstLoadActFuncSet(
                name=nc.get_next_instruction_name(),
                ins=[],
                outs=[],
                act_func_set_id=_set_names.index("natural_log_exp_and_others"),
            )
            _tl.engine = mybir.EngineType.Activation
            nc.scalar.add_instruction(_tl)

            nc.vector.memset(eps_sb[:], EPS)
            nc.vector.memset(vt[:, :, :, CH : CH + 1], 1.0)

            # PE p-state warmup: the Tensor engine needs ~3us of continuous
            # execution to reach peak clock. Junk matmuls fill the idle time
            # while x loads / GN1 stats run, so real matmuls start at speed.
            jtile = big.tile([P, P], bf16, tag="jtile")
            nc.vector.memset(jtile[:], 0.0)

            def warmup(n, tag="t"):
                psj = ps.tile([P, P], f32, tag=tag, bufs=(1 if tag == "t" else 2), name="psj")
                for _ in range(n):
                    nc.tensor.matmul(psj[:], jtile[:], jtile[:], start=True, stop=True)

            warmup(50)

            # ---------- GroupNorm helpers (from baseline) ----------
            def gn_chunk(src_ap, mvs, k):
                """mean/var of one channel chunk -> mvs[:, k, :]."""
                resh = src_ap.rearrange("p (s f) -> p s f", f=512)
                stats = tmpp.tile([P, 2, 6], f32, tag="bnstats")
                for si in range(2):
                    nc.vector.bn_stats(out=stats[:, si, :], in_=resh[:, si, :])
                nc.vector.bn_aggr(out=mvs[:, k, :], in_=stats[:])

            def gn_stat2(mvs, stat2, k, bias=None):
                """stat2[:, k] = (mean, E[x^2]) for chunk k; `bias` shifts the
                mean (stats taken pre-bias); variance is bias-invariant."""
                mean = mvs[:, k, 0:1]
                if bias is not None:
                    meant = tmpp.tile([P, 1], f32, tag="meant")
                    nc.vector.tensor_tensor(meant[:], mean, bias, OP.add)
                    mean = meant[:]
                musq = tmpp.tile([P, 1], f32, tag="musq")
                nc.vector.tensor_tensor(musq[:], mean, mean, OP.mult)
                nc.vector.tensor_tensor(stat2[:, k, 1:2], mvs[:, k, 1:2], musq[:], OP.add)
                nc.vector.tensor_copy(out=stat2[:, k, 0:1], in_=mean)

            def gn_finish(stat2, gs, gb, ab, uniq, psg_tag="c"):
                """Per-channel (A, B) for y = src*A + B from 32-group stats."""
                ps_g = ps.tile([P, 512], f32, tag=psg_tag, bufs=2, name="ps_g")
                for k in range(KC):
                    nc.tensor.matmul(
                        ps_g[0:GROUPS, 0:2], sel_sb[:, k, :], stat2[:, k, :],
                        start=(k == 0), stop=(k == KC - 1),
                    )
                gstat = big.tile([GROUPS, 2], f32, tag=f"gstat{uniq}")
                gms = tmpp.tile([GROUPS, 2], f32, tag="gms")
                nc.vector.tensor_copy(out=gms[:], in_=ps_g[0:GROUPS, 0:2])
                nc.vector.tensor_copy(out=gstat[:, 0:1], in_=gms[:, 0:1])
                gvar = tmpp.tile([GROUPS, 1], f32, tag="gvar")
                gmusq = tmpp.tile([GROUPS, 1], f32, tag="gmusq")
                nc.vector.tensor_tensor(gmusq[:], gms[:, 0:1], gms[:, 0:1], OP.mult)
                nc.vector.tensor_tensor(gvar[:], gms[:, 1:2], gmusq[:], OP.subtract)
                nc.scalar.activation(out=gvar[:], in_=gvar[:], func=AF.Ln, bias=eps_sb[:])
                nc.scalar.activation(out=gstat[:, 1:2], in_=gvar[:], func=AF.Exp, scale=-0.5)
                ps_c = ps.tile([P, 512], f32, tag="c", bufs=2)
                for k in range(KC):
                    nc.tensor.matmul(
                        ps_c[:, k * 2 : k * 2 + 2], selt_sb[:, k, :], gstat[:],
                        start=True, stop=True,
                    )
                # read (mean, rstd) straight from psum: each op has only one
                # PSUM operand, so no staging copy is needed
                psc_v = ps_c[:, 0 : 2 * KC].rearrange("p (k c) -> p k c", c=2)
                nc.vector.tensor_tensor(ab[:, :, 0], gs[:, :], psc_v[:, :, 1], OP.mult)
                ma = tmpp.tile([P, KC], f32, tag="ma")
                nc.vector.tensor_tensor(ma[:], psc_v[:, :, 0], ab[:, :, 0], OP.mult)
                nc.vector.tensor_tensor(ab[:, :, 1], gb[:, :], ma[:], OP.subtract)

            # ---------- GN1 -> hn (bf16) ----------
            mvs1 = tmpp.tile([P, KC, 2], f32, tag="mvs")
            stat21 = big.tile([P, KC, 2], f32, tag="stat21")
            for k in range(KC):
                gn_chunk(x_sb[:, k, :], mvs1, k)
                gn_stat2(mvs1, stat21, k)
            gn_finish(stat21, g1s, g1b, ab1, "1")
            warmup(20)
            # split by column half so the first q/k conv tiles can start as
            # soon as the t0 halves of all four chunks are normalized; the t1
            # halves go to the (still idle) ACT engine so DVE can move on to
            # the conv evictions sooner
            for k in range(KC):
                nc.vector.tensor_scalar(
                    hn[:, k, 0:512], x_sb[:, k, 0:512],
                    ab1[:, k, 0:1], ab1[:, k, 1:2], OP.mult, OP.add,
                )
            for k in range(KC):
                nc.scalar.activation(
                    out=hn[:, k, 512:1024], in_=x_sb[:, k, 512:1024],
                    func=AF.Identity, scale=ab1[:, k, 0:1], bias=ab1[:, k, 1:2],
                )

            # ---------- conv helpers ----------
            def qk_conv(m, first=False):
                # t-major (q t0, k t0, q t1, k t1) so head (m,0)'s first
                # S^T chunks have their operands as early as possible; for
                # the very first conv the t0 evictions ride the still-idle
                # ACT engine so DVE isn't the serial gate to the first exp
                for t in range(NT):
                    psq = ps.tile([P, 512], f32, tag="c", bufs=2)
                    for k in range(KC):
                        nc.tensor.matmul(
                            psq[:, :], wq_sb[:, m, k, :],
                            hn[:, k, t * 512 : (t + 1) * 512],
                            start=(k == 0), stop=(k == KC - 1),
                        )
                    if first and t == 0:
                        nc.scalar.activation(
                            out=qbt[:, m, t, :], in_=psq[:],
                            func=AF.Identity, bias=bq_sb[:, m : m + 1],
                        )
                    else:
                        nc.vector.tensor_scalar(
                            qbt[:, m, t, :], psq[:], bq_sb[:, m : m + 1], None, OP.add,
                        )
                    psk = ps.tile([P, 512], f32, tag="c", bufs=2)
                    for k in range(KC):
                        nc.tensor.matmul(
                            psk[:, :], wk_sb[:, m, k, :],
                            hn[:, k, t * 512 : (t + 1) * 512],
                            start=(k == 0), stop=(k == KC - 1),
                        )
                    if first and t == 0:
                        nc.scalar.activation(
                            out=kbt[:, m, 4 * t : 4 * t + 4, :],
                            in_=psk[:].rearrange("p (a b) -> p a b", a=4),
                            func=AF.Identity, bias=bk_sb[:, m : m + 1],
                        )
                    else:
                        nc.vector.tensor_scalar(
                            kbt[:, m, 4 * t : 4 * t + 4, :],
                            psk[:].rearrange("p (a b) -> p a b", a=4),
                            bk_sb[:, m : m + 1], None, OP.add,
                        )

            def v_conv(nt):
                psv = ps.tile([P, 512], f32, tag="c", bufs=2)
                for k in range(KC):
                    nc.tensor.matmul(
                        psv[:, :], hn[:, k, nt * P : (nt + 1) * P], wv_sb[:, k, :],
                        start=(k == 0), stop=(k == KC - 1),
                    )
                nc.vector.tensor_copy(
                    out=vt[:, nt, :, 0:CH],
                    in_=psv[:].rearrange("p (h c) -> p h c", h=NHEAD),
                )

            # ---------- attention helpers ----------
            def s_head(h):
                """S^T (bf16) + exp per s-chunk; returns exps buf."""
                p, e = h // 2, h % 2
                rows = slice(64 * e, 64 * e + 64)
                eb = expp.tile([P, SC, N], bf16, tag="exps")
                for sc in range(SC):
                    ps_st = ps.tile([P, N], f32, tag="s", bufs=2)
                    for t in range(NT):
                        nc.tensor.matmul(
                            ps_st[:, t * 512 : (t + 1) * 512],
                            kbt[rows, p, sc, :],
                            qbt[rows, p, t, :],
                            start=True, stop=True,
                        )
                        if h == 0 and sc == 0:
                            # half-granular first exp: ACT starts as soon as
                            # the very first S tile lands
                            tsl = slice(t * 512, (t + 1) * 512)
                            nc.scalar.activation(
                                out=eb[:, sc, tsl], in_=ps_st[:, tsl],
                                func=AF.Exp, scale=EXPSCALE,
                            )
                    if not (h == 0 and sc == 0):
                        nc.scalar.activation(
                            out=eb[:, sc, :], in_=ps_st[:], func=AF.Exp, scale=EXPSCALE,
                        )
                return eb

            def a_head(h, eb, tag="a"):
                """A^T: out[t, c] += exps[s, t] v[s, c]; normalize by the
                ones-column denominator (per-partition) on eviction.
                With tag != "a" (two free slots) both halves' matmuls are
                emitted back-to-back so the evictions pipeline."""
                aTt = atp.tile([P, TC, CH], bf16, tag="aT")
                rc = atp.tile([P, TC, 1], f32, tag="rc")
                tiles = []
                for half in range(2):
                    ps_a = ps.tile([P, 4, CH + 1], f32, tag=tag, bufs=(1 if tag == "a" else 2), name="ps_a")
                    tiles.append(ps_a)
                    for j in range(4):
                        tcc = 4 * half + j
                        for sc in range(SC):
                            nc.tensor.matmul(
                                ps_a[:, j, :],
                                eb[:, sc, tcc * P : (tcc + 1) * P],
                                vt[:, sc, h, :],
                                start=(sc == 0), stop=(sc == SC - 1),
                            )
                    if tag == "a":
                        hsl = slice(4 * half, 4 * half + 4)
                        nc.vector.reciprocal(rc[:, hsl, :], ps_a[:, :, CH : CH + 1])
                        nc.vector.tensor_tensor(
                            aTt[:, hsl, :], ps_a[:, :, 0:CH],
                            rc[:, hsl, :].to_broadcast((P, 4, CH)), OP.mult,
                        )
                if tag != "a":
                    for half in range(2):
                        hsl = slice(4 * half, 4 * half + 4)
                        nc.vector.reciprocal(rc[:, hsl, :], tiles[half][:, :, CH : CH + 1])
                        nc.vector.tensor_tensor(
                            aTt[:, hsl, :], tiles[half][:, :, 0:CH],
                            rc[:, hsl, :].to_broadcast((P, 4, CH)), OP.mult,
                        )
                return aTt

            def t_head(h, aTt, ps_tt):
                e = h % 2
                for tcc in range(TC):
                    nc.tensor.transpose(
                        ps_tt[64 * e : 64 * e + 64, tcc, :], aTt[:, tcc, :], ident[:],
                    )

            def t_evict(p, ps_tt):
                # split in t-halves so consumers of the first half start early
                for half in range(2):
                    nc.vector.tensor_scalar(
                        a_u[:, p, half * 512 : (half + 1) * 512].rearrange(
                            "p (a b) -> p a b", a=TC // 2
                        ),
                        ps_tt[:, 4 * half : 4 * half + 4, :],
                        bv_sb[:, p : p + 1], None, OP.add,
                    )

            # ---------- main attention pipeline ----------
            qk_conv(0)
            eb0 = s_head(0)
            # v-convs interleaved with head 1's S chunks: total PE work in
            # the first two heads' exp window barely fits, so spread it
            p1, e1 = 0, 1
            rows1 = slice(64, 128)
            eb1 = expp.tile([P, SC, N], bf16, tag="exps", name="eb1")
            for sc in range(SC):
                ps_st = ps.tile([P, N], f32, tag="s", bufs=2, name="ps_st")
                for t in range(NT):
                    nc.tensor.matmul(
                        ps_st[:, t * 512 : (t + 1) * 512],
                        kbt[rows1, 0, sc, :], qbt[rows1, 0, t, :],
                        start=True, stop=True,
                    )
                nc.scalar.activation(
                    out=eb1[:, sc, :], in_=ps_st[:], func=AF.Exp, scale=EXPSCALE,
                )
                v_conv(sc)
            qk_conv(1)
            nc.sync.dma_start(out_d[:], x_d[:])  # residual prefill
            eb2 = s_head(2)
            ebs = {0: eb0, 1: eb1, 2: eb2}
            ps_tts = {}

            def pair_tail(h, tag="a"):
                # A + transposes for head h, then evict the pair tile
                p, e = h // 2, h % 2
                aTt = a_head(h, ebs.pop(h), tag)
                if e == 0:
                    ps_tts[p] = ps.tile([P, TC, P], bf16, tag="t", bufs=1, name="ps_tt")
                t_head(h, aTt, ps_tts[p])
                if e == 1:
                    t_evict(p, ps_tts.pop(p))

            pair_tail(0)
            ebs[3] = s_head(3)
            pair_tail(1)
            qk_conv(2)
            ebs[4] = s_head(4)
            pair_tail(2)
            ebs[5] = s_head(5)
            pair_tail(3)
            qk_conv(3)
            ebs[6] = s_head(6)
            pair_tail(4)
            ebs[7] = s_head(7)
            pair_tail(5)
            # proj partial sums over the finished chunks (pairs 0..2) run
            # under the last heads' exps (in the conv psum slots, which are
            # free by now); only the k=3 contraction is left for the tail
            partial = big.tile([P, KC, N], bf16, tag="partial")
            for m in range(KC):
                for t in range(NT):
                    tsl = slice(t * 512, (t + 1) * 512)
                    psw = ps.tile([P, 512], f32, tag="c", bufs=2)
                    for k in range(KC - 1):
                        nc.tensor.matmul(
                            psw[:, :], wp_sb[:, k, m * P : (m + 1) * P],
                            a_u[:, k, tsl],
                            start=(k == 0), stop=(k == KC - 2),
                        )
                    nc.vector.tensor_copy(out=partial[:, m, tsl], in_=psw[:])
            pair_tail(6)
            # Pre-open the proj psum groups with the `partial` identity-adds
            # while the last head's exps run; only the k=3 matmuls (gated on
            # pair 3's a_u) remain for the tail. Slot map: m0/m1 in the two
            # "s" slots (free as the last exps drain), m2 in "a"+"t", m3 in
            # the "c" slots (after head 7's A halves).
            tailps = []
            for m in range(2):
                psp = ps.tile([P, N], f32, tag="s", bufs=2, name="psp")
                tailps.append([psp[:, 0:512], psp[:, 512:1024]])
                for t in range(NT):
                    nc.tensor.matmul(
                        tailps[m][t], ident[:], partial[:, m, t * 512 : (t + 1) * 512],
                        start=True, stop=False,
                    )
            pair_tail(7, tag="c")
            ph2a = ps.tile([P, 512], f32, tag="a", bufs=1, name="ph2a")
            ph2b = ps.tile([P, 512], f32, tag="t", bufs=1, name="ph2b")
            ph3a = ps.tile([P, 512], f32, tag="c", bufs=2, name="ph3a")
            ph3b = ps.tile([P, 512], f32, tag="c", bufs=2, name="ph3b")
            tailps.append([ph2a[:, :], ph2b[:, :]])
            tailps.append([ph3a[:, :], ph3b[:, :]])
            for m in (2, 3):
                for t in range(NT):
                    nc.tensor.matmul(
                        tailps[m][t], ident[:], partial[:, m, t * 512 : (t + 1) * 512],
                        start=True, stop=False,
                    )

            if debug_taps:
                nc.sync.dma_start(dbg["d_hn"][:], hn[:])
                nc.sync.dma_start(dbg["d_q8"][:], qbt[:])
                nc.sync.dma_start(dbg["d_k8"][:], kbt[:])
                nc.sync.dma_start(dbg["d_vt"][:], vt[:])
                nc.sync.dma_start(dbg["d_e0"][:], eb0[:])
                nc.sync.dma_start(dbg["d_au"][:], a_u[:])

            # ---------- proj tail: k=3 matmuls + stats + ACT evictions ----------
            mvs2 = tmpp.tile([P, KC, 2], f32, tag="mvs")
            stat22 = big.tile([P, KC, 2], f32, tag="stat22")
            for m in range(KC):
                halves = tailps[m]
                statp = tmpp.tile([P, 2, 6], f32, tag="bnstats")
                for t in range(NT):
                    tsl = slice(t * 512, (t + 1) * 512)
                    nc.tensor.matmul(
                        halves[t], wp_sb[:, KC - 1, m * P : (m + 1) * P],
                        a_u[:, KC - 1, tsl], start=False, stop=True,
                    )
                    nc.vector.bn_stats(out=statp[:, t, :], in_=halves[t])
                nc.vector.bn_aggr(out=mvs2[:, m, :], in_=statp[:])
                gn_stat2(mvs2, stat22, m, bias=bp_sb[:, m : m + 1])
                # bias-add eviction on the (tail-idle) ACT engine so the DVE
                # stats chain isn't also carrying the evictions
                for t in range(NT):
                    nc.scalar.activation(
                        out=proj[:, m, t * 512 : (t + 1) * 512], in_=halves[t],
                        func=AF.Identity, bias=bp_sb[:, m : m + 1],
                    )

            if debug_taps:
                nc.sync.dma_start(dbg["d_pj"][:], proj[:])

            # ---------- GN2 + residual (via bf16 DMA accumulate) ----------
            gn_finish(stat22, g2s, g2b, ab2, "2", psg_tag="s")
            for k in range(KC):
                nc.vector.tensor_scalar(
                    stg[:, k, :], proj[:, k, :],
                    ab2[:, k, 0:1], ab2[:, k, 1:2], OP.mult, OP.add,
                )
                if k % 2 == 1:
                    nc.gpsimd.dma_start(
                        out_d[:, k - 1 : k + 1, :], stg[:, k - 1 : k + 1, :],
                        accum_op=OP.add,
                    )

    nc.compile()
    return nc


def _host_prep(x, gn1_scale, gn1_bias, w_qkv, b_qkv, w_proj, b_proj, gn2_scale, gn2_bias):
    """Build per-core input maps (numpy only)."""
    import ml_dtypes

    f = np.float32
    bf = ml_dtypes.bfloat16
    x = np.asarray(x, f)
    w_qkv = np.asarray(w_qkv, f)
    b_qkv = np.asarray(b_qkv, f)
    w_proj = np.asarray(w_proj, f)
    b_proj = np.asarray(b_proj, f)
    gn1_scale = np.asarray(gn1_scale, f)
    gn1_bias = np.asarray(gn1_bias, f)
    gn2_scale = np.asarray(gn2_scale, f)
    gn2_bias = np.asarray(gn2_bias, f)

    def chunk_vec(v):  # [C] -> [P, KC]
        return np.ascontiguousarray(v.reshape(KC, P).T)

    def chunk_mat(wt):  # [C, M] -> [P, KC, M]
        return np.ascontiguousarray(wt.reshape(KC, P, -1).transpose(1, 0, 2))

    def mtile(w):  # [P, KC, C] -> [P, M, KC, P]
        return np.ascontiguousarray(w.reshape(P, KC, KC, P).transpose(0, 2, 1, 3))

    idx = np.arange(NHEAD)[:, None] * (3 * CH) + np.arange(CH)[None, :]
    q_idx, k_idx, v_idx = idx.ravel(), (idx + CH).ravel(), (idx + 2 * CH).ravel()

    s4 = float(CH) ** -0.25
    wq = mtile(chunk_mat(w_qkv[q_idx].T * (s4 * ALPHA)))
    wk = mtile(chunk_mat(w_qkv[k_idx].T * (s4 * ALPHA)))
    wv = chunk_mat(w_qkv[v_idx].T)
    wp = chunk_mat(w_proj.T).astype(bf)
    biases = np.stack(
        [
            chunk_vec(b_qkv[q_idx] * (s4 * ALPHA)),
            chunk_vec(b_qkv[k_idx] * (s4 * ALPHA)),
            chunk_vec(b_qkv[v_idx]),
            chunk_vec(b_proj),
            chunk_vec(gn1_scale), chunk_vec(gn1_bias),
            chunk_vec(gn2_scale), chunk_vec(gn2_bias),
        ],
        axis=1,
    )

    cidx = np.arange(C)
    sel = np.zeros((P, KC, GROUPS), f)
    sel[cidx % P, cidx // P, cidx // 16] = 1.0 / 16.0
    selt = np.zeros((GROUPS, KC, P), f)
    selt[cidx // 16, cidx // P, cidx % P] = 1.0
    ident = np.eye(P, dtype=bf)

    shared = {
        "wq": wq, "wk": wk, "wv": wv, "wp": wp,
        "biases": np.ascontiguousarray(biases),
        "sel": sel, "selt": selt, "ident": ident,
    }
    in_maps = []
    for b in range(B):
        xb = np.ascontiguousarray(
            x[b].reshape(C, N).reshape(KC, P, N).transpose(1, 0, 2)
        ).astype(bf)
        in_maps.append({"x": xb, **shared})
    return in_maps


def _assemble(results):
    out = np.empty((B, C, H, W), np.float32)
    for b in range(B):
        ob = np.asarray(results[b]["out"]).astype(np.float32)  # [P, KC, N]
        out[b] = ob.transpose(1, 0, 2).reshape(C, N).reshape(C, H, W)
    return out


def get_nc():
    if "nc" not in _CACHE:
        _CACHE["nc"] = _build_nc()
    return _CACHE["nc"]


def kernel(x, gn1_scale, gn1_bias, w_qkv, b_qkv, w_proj, b_proj, gn2_scale, gn2_bias):
    from concourse.bass_utils import run_bass_kernel_spmd

    nc = get_nc()
    in_maps = _host_prep(
        x, gn1_scale, gn1_bias, w_qkv, b_qkv, w_proj, b_proj, gn2_scale, gn2_bias
    )
    res = run_bass_kernel_spmd(nc, in_maps, core_ids=list(range(B)))
    return _assemble(res.results)


# revision 82
# speedup vs baseline: 1.2989x; 1.0011x over previous
"""Attention2d Trainium2 Bass kernel (v2).

Reference computation (per batch element b of 8, one NeuronCore each):
    hn  = GroupNorm32(x) * gn1_scale + gn1_bias
    qkv = w_qkv @ hn + b_qkv          (1x1 conv == matmul over channels)
    per head h (8 heads, ch=64): wgt = softmax((q*s)^T (k*s)), s = ch**-0.25
    a   = v @ wgt^T
    out = GroupNorm32(w_proj @ a + b_proj)
    y   = x + out

Device strategy (data-parallel over batch, 1 core per batch element):
  - channel-chunked layouts [128, 4, N] (partition = channel % 128-chunk)
  - x and out in bf16 (halves the HBM head/tail); hn kept f32 so the v-path
    sees only one rounding; q,k,v convs in f32r (full PE rate at free>=256)
  - q,k evicted to bf16 with a symmetric alpha*ch^-0.25 prescale folded into
    the weights; S^T = k^T q in bf16; exp on ACT with scale 1/alpha^2
  - exps stored bf16, triple-buffered so ACT never waits on the A-side
  - A computed TRANSPOSED: out[t, c] = sum_s exps[s, t] v[s, c]; the softmax
    denominator rides along as a ones-column of v^T and lands per-partition,
    so normalization is one DVE broadcast-divide on eviction
  - a^T transposed back to channel-major via PE identity-transposes (bf16)
  - proj split: k=0..2 partial-summed under the last heads' exps (re-injected
    at the tail via identity matmuls into pre-opened psum groups), GroupNorm2
    stats read psum directly on DVE while ACT does the bias-add evictions,
    residual added by a bf16 DMA-accumulate against out[] prefilled with x
  - PE p-state warmup: junk matmuls keep the Tensor engine continuously busy
    through the prologue so real matmuls start at peak clock
"""

import numpy as np

NHEAD = 8
GROUPS = 32
EPS = 1e-5
B, C, H, W = 8, 512, 32, 32
N = H * W            # 1024 spatial positions
CH = C // NHEAD      # 64 channels per head
P = 128              # partitions
KC = C // P          # 4 channel chunks
NT = N // 512        # 2 column tiles of 512
SC = N // P          # 8 s-chunks
TC = N // P          # 8 t-chunks
ALPHA = 8.0          # fp8 prescale per side (folded into wq/wk, bq/bk)
EXPSCALE = 1.0 / (ALPHA * ALPHA)

_CACHE = {}


def _build_nc(debug_taps=False):
    import concourse.tile as tile
    from concourse import mybir, bacc
    from concourse.hw_specs import get_activation_tables

    f32 = mybir.dt.float32
    f32r = mybir.dt.float32r
    bf16 = mybir.dt.bfloat16
    AF = mybir.ActivationFunctionType
    OP = mybir.AluOpType
    DR = mybir.MatmulPerfMode.DoubleRow

    nc = bacc.Bacc("TRN2", target_bir_lowering=False, num_devices=8)

    x_d = nc.dram_tensor("x", [P, KC, N], bf16, kind="ExternalInput")
    wq_d = nc.dram_tensor("wq", [P, KC, KC, P], f32r, kind="ExternalInput")
    wk_d = nc.dram_tensor("wk", [P, KC, KC, P], f32r, kind="ExternalInput")
    wv_d = nc.dram_tensor("wv", [P, KC, C], f32r, kind="ExternalInput")
    wp_d = nc.dram_tensor("wp", [P, KC, C], bf16, kind="ExternalInput")
    # bq, bk, bv, bp, g1s, g1b, g2s, g2b packed as one tensor (one DMA —
    # HWDGE dispatch is ~625ns per DMA and the prologue is latency-critical)
    bias_d = nc.dram_tensor("biases", [P, 8, KC], f32, kind="ExternalInput")
    sel_d = nc.dram_tensor("sel", [P, KC, GROUPS], f32, kind="ExternalInput")
    selt_d = nc.dram_tensor("selt", [GROUPS, KC, P], f32, kind="ExternalInput")
    id_d = nc.dram_tensor("ident", [P, P], bf16, kind="ExternalInput")
    out_d = nc.dram_tensor("out", [P, KC, N], bf16, kind="ExternalOutput")
    if debug_taps:
        dbg = {
            "d_hn": nc.dram_tensor("d_hn", [P, KC, N], f32, kind="ExternalOutput"),
            "d_q8": nc.dram_tensor("d_q8", [P, KC, NT, 512], bf16, kind="ExternalOutput"),
            "d_k8": nc.dram_tensor("d_k8", [P, KC, SC, P], bf16, kind="ExternalOutput"),
            "d_vt": nc.dram_tensor("d_vt", [P, SC, NHEAD, CH + 1], bf16, kind="ExternalOutput"),
            "d_e0": nc.dram_tensor("d_e0", [P, SC, N], bf16, kind="ExternalOutput"),
            "d_au": nc.dram_tensor("d_au", [P, KC, N], bf16, kind="ExternalOutput"),
            "d_pj": nc.dram_tensor("d_pj", [P, KC, N], bf16, kind="ExternalOutput"),
        }

    with tile.TileContext(nc) as tc:
        with (
            tc.tile_pool(name="big", bufs=1) as big,
            tc.tile_pool(name="expp", bufs=3) as expp,
            tc.tile_pool(name="atp", bufs=2) as atp,
            tc.tile_pool(name="tmpp", bufs=2) as tmpp,
            tc.tile_pool(name="ps", bufs=1, space="PSUM") as ps,
        ):
            # ---------- persistent SBUF tiles ----------
            x_sb = big.tile([P, KC, N], bf16, tag="x_sb")
            hn = big.tile([P, KC, N], f32r, tag="hn")
            qbt = big.tile([P, KC, NT, 512], bf16, tag="qbt")
            kbt = big.tile([P, KC, SC, P], bf16, tag="kbt")
            vt = big.tile([P, SC, NHEAD, CH + 1], bf16, tag="vt")
            a_u = big.tile([P, KC, N], bf16, tag="a_u")
            proj = big.tile([P, KC, N], bf16, tag="proj")
            stg = big.tile([P, KC, N], bf16, tag="stg")
            wq_sb = big.tile([P, KC, KC, P], f32r, tag="wq_sb")
            wk_sb = big.tile([P, KC, KC, P], f32r, tag="wk_sb")
            wv_sb = big.tile([P, KC, C], f32r, tag="wv_sb")
            wp_sb = big.tile([P, KC, C], bf16, tag="wp_sb")
            ident = big.tile([P, P], bf16, tag="ident")
            sel_sb = big.tile([P, KC, GROUPS], f32, tag="sel_sb")
            selt_sb = big.tile([GROUPS, KC, P], f32, tag="selt_sb")
            bias_sb = big.tile([P, 8, KC], f32, tag="bias_sb")
            bq_sb = bias_sb[:, 0]
            bk_sb = bias_sb[:, 1]
            bv_sb = bias_sb[:, 2]
            bp_sb = bias_sb[:, 3]
            g1s = bias_sb[:, 4]
            g1b = bias_sb[:, 5]
            g2s = bias_sb[:, 6]
            g2b = bias_sb[:, 7]
            eps_sb = big.tile([GROUPS, 1], f32, tag="eps_sb")
            ab1 = big.tile([P, KC, 2], f32, tag="ab1")
            ab2 = big.tile([P, KC, 2], f32, tag="ab2")

            # ---------- prologue ----------
            # x DMAs small-first so bn_stats start early, coarser later so
            # HWDGE dispatch (625ns/DMA) doesn't serialize the prologue
            nc.sync.dma_start(x_sb[:, 0, 0:512], x_d[:, 0, 0:512])
            nc.sync.dma_start(x_sb[:, 0, 512:1024], x_d[:, 0, 512:1024])
            nc.sync.dma_start(x_sb[:, 1, :], x_d[:, 1, :])
            nc.sync.dma_start(x_sb[:, 2, :], x_d[:, 2, :])
            nc.sync.dma_start(x_sb[:, 3, 0:512], x_d[:, 3, 0:512])
            nc.sync.dma_start(x_sb[:, 3, 512:1024], x_d[:, 3, 512:1024])
            for sb, d in (
                (wq_sb, wq_d), (sel_sb, sel_d), (selt_sb, selt_d),
                (bias_sb, bias_d), (wk_sb, wk_d), (ident, id_d),
                (wv_sb, wv_d), (wp_sb, wp_d),
            ):
                nc.sync.dma_start(sb[:], d[:])

            # Preload the combined ln+exp ACT table set once (avoids ~1.3us
            # table swaps between Ln and Exp at every GN <-> attention switch)
            _set_names = list(get_activation_tables(nc.m.arch).keys())
            _tl = mybir.InstLoadActFuncSet(
                name=nc.get_next_instruction_name(),
                ins=[],
                outs=[],
                act_func_set_id=_set_names.index("natural_log_exp_and_others"),
            )
            _tl.engine = mybir.EngineType.Activation
            nc.scalar.add_instruction(_tl)

            nc.vector.memset(eps_sb[:], EPS)
            nc.vector.memset(vt[:, :, :, CH : CH + 1], 1.0)

            # PE p-state warmup: the Tensor engine needs ~3us of continuous
            # execution to reach peak clock. Junk matmuls fill the idle time
            # while x loads / GN1 stats run, so real matmuls start at speed.
            jtile = big.tile([P, P], bf16, tag="jtile")
            nc.vector.memset(jtile[:], 0.0)

            def warmup(n, tag="t"):
                psj = ps.tile([P, P], f32, tag=tag, bufs=(1 if tag == "t" else 2), name="psj")
                for _ in range(n):
                    nc.tensor.matmul(psj[:], jtile[:], jtile[:], start=True, stop=True)

            warmup(50)

            # ---------- GroupNorm helpers (from baseline) ----------
            def gn_chunk(src_ap, mvs, k):
                """mean/var of one channel chunk -> mvs[:, k, :]."""
                resh = src_ap.rearrange("p (s f) -> p s f", f=512)
                stats = tmpp.tile([P, 2, 6], f32, tag="bnstats")
                for si in range(2):
                    nc.vector.bn_stats(out=stats[:, si, :], in_=resh[:, si, :])
                nc.vector.bn_aggr(out=mvs[:, k, :], in_=stats[:])

            def gn_stat2(mvs, stat2, k, bias=None):
                """stat2[:, k] = (mean, E[x^2]) for chunk k; `bias` shifts the
                mean (stats taken pre-bias); variance is bias-invariant."""
                mean = mvs[:, k, 0:1]
                if bias is not None:
                    meant = tmpp.tile([P, 1], f32, tag="meant")
                    nc.vector.tensor_tensor(meant[:], mean, bias, OP.add)
                    mean = meant[:]
                musq = tmpp.tile([P, 1], f32, tag="musq")
                nc.vector.tensor_tensor(musq[:], mean, mean, OP.mult)
                nc.vector.tensor_tensor(stat2[:, k, 1:2], mvs[:, k, 1:2], musq[:], OP.add)
                nc.vector.tensor_copy(out=stat2[:, k, 0:1], in_=mean)

            def gn_finish(stat2, gs, gb, ab, uniq, psg_tag="c"):
                """Per-channel (A, B) for y = src*A + B from 32-group stats."""
                ps_g = ps.tile([P, 512], f32, tag=psg_tag, bufs=2, name="ps_g")
                for k in range(KC):
                    nc.tensor.matmul(
                        ps_g[0:GROUPS, 0:2], sel_sb[:, k, :], stat2[:, k, :],
                        start=(k == 0), stop=(k == KC - 1),
                    )
                gstat = big.tile([GROUPS, 2], f32, tag=f"gstat{uniq}")
                gms = tmpp.tile([GROUPS, 2], f32, tag="gms")
                nc.vector.tensor_copy(out=gms[:], in_=ps_g[0:GROUPS, 0:2])
                nc.vector.tensor_copy(out=gstat[:, 0:1], in_=gms[:, 0:1])
                gvar = tmpp.tile([GROUPS, 1], f32, tag="gvar")
                gmusq = tmpp.tile([GROUPS, 1], f32, tag="gmusq")
                nc.vector.tensor_tensor(gmusq[:], gms[:, 0:1], gms[:, 0:1], OP.mult)
                nc.vector.tensor_tensor(gvar[:], gms[:, 1:2], gmusq[:], OP.subtract)
                nc.scalar.activation(out=gvar[:], in_=gvar[:], func=AF.Ln, bias=eps_sb[:])
                nc.scalar.activation(out=gstat[:, 1:2], in_=gvar[:], func=AF.Exp, scale=-0.5)
                ps_c = ps.tile([P, 512], f32, tag="c", bufs=2)
                for k in range(KC):
                    nc.tensor.matmul(
                        ps_c[:, k * 2 : k * 2 + 2], selt_sb[:, k, :], gstat[:],
                        start=True, stop=True,
                    )
                # read (mean, rstd) straight from psum: each op has only one
                # PSUM operand, so no staging copy is needed
                psc_v = ps_c[:, 0 : 2 * KC].rearrange("p (k c) -> p k c", c=2)
                nc.vector.tensor_tensor(ab[:, :, 0], gs[:, :], psc_v[:, :, 1], OP.mult)
                ma = tmpp.tile([P, KC], f32, tag="ma")
                nc.vector.tensor_tensor(ma[:], psc_v[:, :, 0], ab[:, :, 0], OP.mult)
                nc.vector.tensor_tensor(ab[:, :, 1], gb[:, :], ma[:], OP.subtract)

            # ---------- GN1 -> hn (bf16) ----------
            mvs1 = tmpp.tile([P, KC, 2], f32, tag="mvs")
            stat21 = big.tile([P, KC, 2], f32, tag="stat21")
            for k in range(KC):
                gn_chunk(x_sb[:, k, :], mvs1, k)
                gn_stat2(mvs1, stat21, k)
            gn_finish(stat21, g1s, g1b, ab1, "1")
            warmup(20)
            # split by column half so the first q/k conv tiles can start as
            # soon as the t0 halves of all four chunks are normalized; the t1
            # halves go to the (still idle) ACT engine so DVE can move on to
            # the conv evictions sooner
            for k in range(KC):
                nc.vector.tensor_scalar(
                    hn[:, k, 0:512], x_sb[:, k, 0:512],
                    ab1[:, k, 0:1], ab1[:, k, 1:2], OP.mult, OP.add,
                )
            for k in range(KC):
                nc.scalar.activation(
                    out=hn[:, k, 512:1024], in_=x_sb[:, k, 512:1024],
                    func=AF.Identity, scale=ab1[:, k, 0:1], bias=ab1[:, k, 1:2],
                )

            # ---------- conv helpers ----------
            def qk_conv(m, first=False):
                # t-major (q t0, k t0, q t1, k t1) so head (m,0)'s first
                # S^T chunks have their operands as early as possible; for
                # the very first conv the t0 evictions ride the still-idle
                # ACT engine so DVE isn't the serial gate to the first exp
                for t in range(NT):
                    psq = ps.tile([P, 512], f32, tag="c", bufs=2)
                    for k in range(KC):
                        nc.tensor.matmul(
                            psq[:, :], wq_sb[:, m, k, :],
                            hn[:, k, t * 512 : (t + 1) * 512],
                            start=(k == 0), stop=(k == KC - 1),
                        )
                    nc.vector.tensor_scalar(
                        qbt[:, m, t, :], psq[:], bq_sb[:, m : m + 1], None, OP.add,
                    )
                    psk = ps.tile([P, 512], f32, tag="c", bufs=2)
                    for k in range(KC):
                        nc.tensor.matmul(
                            psk[:, :], wk_sb[:, m, k, :],
                            hn[:, k, t * 512 : (t + 1) * 512],
                            start=(k == 0), stop=(k == KC - 1),
                        )
                    if first and t == 0:
                        # sc0 alone first: the head-0 S^T chunk 0 (and hence
                        # the first exp) only needs kbt[...sc0]
                        nc.vector.tensor_scalar(
                            kbt[:, m, 0:1, :],
                            psk[:, 0:128].rearrange("p (a b) -> p a b", a=1),
                            bk_sb[:, m : m + 1], None, OP.add,
                        )
                        nc.vector.tensor_scalar(
                            kbt[:, m, 1:4, :],
                            psk[:, 128:512].rearrange("p (a b) -> p a b", a=3),
                            bk_sb[:, m : m + 1], None, OP.add,
                        )
                    else:
                        nc.vector.tensor_scalar(
                            kbt[:, m, 4 * t : 4 * t + 4, :],
                            psk[:].rearrange("p (a b) -> p a b", a=4),
                            bk_sb[:, m : m + 1], None, OP.add,
                        )

            def v_conv(nt):
                psv = ps.tile([P, 512], f32, tag="c", bufs=2)
                for k in range(KC):
                    nc.tensor.matmul(
                        psv[:, :], hn[:, k, nt * P : (nt + 1) * P], wv_sb[:, k, :],
                        start=(k == 0), stop=(k == KC - 1),
                    )
                nc.vector.tensor_copy(
                    out=vt[:, nt, :, 0:CH],
                    in_=psv[:].rearrange("p (h c) -> p h c", h=NHEAD),
                )

            # ---------- attention helpers ----------
            def s_head(h):
                """S^T (bf16) + exp per s-chunk; returns exps buf."""
                p, e = h // 2, h % 2
                rows = slice(64 * e, 64 * e + 64)
                eb = expp.tile([P, SC, N], bf16, tag="exps")
                for sc in range(SC):
                    ps_st = ps.tile([P, N], f32, tag="s", bufs=2)
                    for t in range(NT):
                        nc.tensor.matmul(
                            ps_st[:, t * 512 : (t + 1) * 512],
                            kbt[rows, p, sc, :],
                            qbt[rows, p, t, :],
                            start=True, stop=True,
                        )
                        if h == 0 and sc == 0:
                            # half-granular first exp: ACT starts as soon as
                            # the very first S tile lands
                            tsl = slice(t * 512, (t + 1) * 512)
                            nc.scalar.activation(
                                out=eb[:, sc, tsl], in_=ps_st[:, tsl],
                                func=AF.Exp, scale=EXPSCALE,
                            )
                    if not (h == 0 and sc == 0):
                        nc.scalar.activation(
                            out=eb[:, sc, :], in_=ps_st[:], func=AF.Exp, scale=EXPSCALE,
                        )
                return eb

            def a_head(h, eb, tag="a"):
                """A^T: out[t, c] += exps[s, t] v[s, c]; normalize by the
                ones-column denominator (per-partition) on eviction.
                With tag != "a" (two free slots) both halves' matmuls are
                emitted back-to-back so the evictions pipeline."""
                aTt = atp.tile([P, TC, CH], bf16, tag="aT")
                rc = atp.tile([P, TC, 1], f32, tag="rc")
                tiles = []
                for half in range(2):
                    ps_a = ps.tile([P, 4, CH + 1], f32, tag=tag, bufs=(1 if tag == "a" else 2), name="ps_a")
                    tiles.append(ps_a)
                    for j in range(4):
                        tcc = 4 * half + j
                        for sc in range(SC):
                            nc.tensor.matmul(
                                ps_a[:, j, :],
                                eb[:, sc, tcc * P : (tcc + 1) * P],
                                vt[:, sc, h, :],
                                start=(sc == 0), stop=(sc == SC - 1),
                            )
                    if tag == "a":
                        hsl = slice(4 * half, 4 * half + 4)
                        nc.vector.reciprocal(rc[:, hsl, :], ps_a[:, :, CH : CH + 1])
                        nc.vector.tensor_tensor(
                            aTt[:, hsl, :], ps_a[:, :, 0:CH],
                            rc[:, hsl, :].to_broadcast((P, 4, CH)), OP.mult,
                        )
                if tag != "a":
                    for half in range(2):
                        hsl = slice(4 * half, 4 * half + 4)
                        nc.vector.reciprocal(rc[:, hsl, :], tiles[half][:, :, CH : CH + 1])
                        nc.vector.tensor_tensor(
                            aTt[:, hsl, :], tiles[half][:, :, 0:CH],
                            rc[:, hsl, :].to_broadcast((P, 4, CH)), OP.mult,
                        )
                return aTt

            def t_head(h, aTt, ps_tt):
                e = h % 2
                for tcc in range(TC):
                    nc.tensor.transpose(
                        ps_tt[64 * e : 64 * e + 64, tcc, :], aTt[:, tcc, :], ident[:],
                    )

            def t_evict(p, ps_tt):
                # split in t-halves so consumers of the first half start early
                for half in range(2):
                    nc.vector.tensor_scalar(
                        a_u[:, p, half * 512 : (half + 1) * 512].rearrange(
                            "p (a b) -> p a b", a=TC // 2
                        ),
                        ps_tt[:, 4 * half : 4 * half + 4, :],
                        bv_sb[:, p : p + 1], None, OP.add,
                    )

            # ---------- main attention pipeline ----------
            qk_conv(0, first=True)
            eb0 = s_head(0)
            # v-convs interleaved with head 1's S chunks: total PE work in
            # the first two heads' exp window barely fits, so spread it
            p1, e1 = 0, 1
            rows1 = slice(64, 128)
            eb1 = expp.tile([P, SC, N], bf16, tag="exps", name="eb1")
            for sc in range(SC):
                ps_st = ps.tile([P, N], f32, tag="s", bufs=2, name="ps_st")
                for t in range(NT):
                    nc.tensor.matmul(
                        ps_st[:, t * 512 : (t + 1) * 512],
                        kbt[rows1, 0, sc, :], qbt[rows1, 0, t, :],
                        start=True, stop=True,
                    )
                nc.scalar.activation(
                    out=eb1[:, sc, :], in_=ps_st[:], func=AF.Exp, scale=EXPSCALE,
                )
                v_conv(sc)
            qk_conv(1)
            nc.sync.dma_start(out_d[:], x_d[:])  # residual prefill
            eb2 = s_head(2)
            ebs = {0: eb0, 1: eb1, 2: eb2}
            ps_tts = {}

            def pair_tail(h, tag="a"):
                # A + transposes for head h, then evict the pair tile
                p, e = h // 2, h % 2
                aTt = a_head(h, ebs.pop(h), tag)
                if e == 0:
                    ps_tts[p] = ps.tile([P, TC, P], bf16, tag="t", bufs=1, name="ps_tt")
                t_head(h, aTt, ps_tts[p])
                if e == 1:
                    t_evict(p, ps_tts.pop(p))

            pair_tail(0)
            ebs[3] = s_head(3)
            pair_tail(1)
            qk_conv(2)
            ebs[4] = s_head(4)
            pair_tail(2)
            ebs[5] = s_head(5)
            pair_tail(3)
            qk_conv(3)
            ebs[6] = s_head(6)
            pair_tail(4)
            ebs[7] = s_head(7)
            pair_tail(5)
            # proj partial sums over the finished chunks (pairs 0..2) run
            # under the last heads' exps (in the conv psum slots, which are
            # free by now); only the k=3 contraction is left for the tail
            partial = big.tile([P, KC, N], bf16, tag="partial")
            for m in range(KC):
                for t in range(NT):
                    tsl = slice(t * 512, (t + 1) * 512)
                    psw = ps.tile([P, 512], f32, tag="c", bufs=2)
                    for k in range(KC - 1):
                        nc.tensor.matmul(
                            psw[:, :], wp_sb[:, k, m * P : (m + 1) * P],
                            a_u[:, k, tsl],
                            start=(k == 0), stop=(k == KC - 2),
                        )
                    nc.vector.tensor_copy(out=partial[:, m, tsl], in_=psw[:])
            pair_tail(6)
            # Pre-open the proj psum groups with the `partial` identity-adds
            # while the last head's exps run; only the k=3 matmuls (gated on
            # pair 3's a_u) remain for the tail. Slot map: m0/m1 in the two
            # "s" slots (free as the last exps drain), m2 in "a"+"t", m3 in
            # the "c" slots (after head 7's A halves).
            tailps = []
            for m in range(2):
                psp = ps.tile([P, N], f32, tag="s", bufs=2, name="psp")
                tailps.append([psp[:, 0:512], psp[:, 512:1024]])
                for t in range(NT):
                    nc.tensor.matmul(
                        tailps[m][t], ident[:], partial[:, m, t * 512 : (t + 1) * 512],
                        start=True, stop=False,
                    )
            pair_tail(7, tag="c")
            ph2a = ps.tile([P, 512], f32, tag="a", bufs=1, name="ph2a")
            ph2b = ps.tile([P, 512], f32, tag="t", bufs=1, name="ph2b")
            ph3a = ps.tile([P, 512], f32, tag="c", bufs=2, name="ph3a")
            ph3b = ps.tile([P, 512], f32, tag="c", bufs=2, name="ph3b")
            tailps.append([ph2a[:, :], ph2b[:, :]])
            tailps.append([ph3a[:, :], ph3b[:, :]])
            for m in (2, 3):
                for t in range(NT):
                    nc.tensor.matmul(
                        tailps[m][t], ident[:], partial[:, m, t * 512 : (t + 1) * 512],
                        start=True, stop=False,
                    )

            if debug_taps:
                nc.sync.dma_start(dbg["d_hn"][:], hn[:])
                nc.sync.dma_start(dbg["d_q8"][:], qbt[:])
                nc.sync.dma_start(dbg["d_k8"][:], kbt[:])
                nc.sync.dma_start(dbg["d_vt"][:], vt[:])
                nc.sync.dma_start(dbg["d_e0"][:], eb0[:])
                nc.sync.dma_start(dbg["d_au"][:], a_u[:])

            # ---------- proj tail: k=3 matmuls + stats + ACT evictions ----------
            mvs2 = tmpp.tile([P, KC, 2], f32, tag="mvs")
            stat22 = big.tile([P, KC, 2], f32, tag="stat22")
            for m in range(KC):
                halves = tailps[m]
                statp = tmpp.tile([P, 2, 6], f32, tag="bnstats")
                for t in range(NT):
                    tsl = slice(t * 512, (t + 1) * 512)
                    nc.tensor.matmul(
                        halves[t], wp_sb[:, KC - 1, m * P : (m + 1) * P],
                        a_u[:, KC - 1, tsl], start=False, stop=True,
                    )
                    nc.vector.bn_stats(out=statp[:, t, :], in_=halves[t])
                nc.vector.bn_aggr(out=mvs2[:, m, :], in_=statp[:])
                gn_stat2(mvs2, stat22, m, bias=bp_sb[:, m : m + 1])
                # bias-add eviction on the (tail-idle) ACT engine so the DVE
                # stats chain isn't also carrying the evictions
                for t in range(NT):
                    nc.scalar.activation(
                        out=proj[:, m, t * 512 : (t + 1) * 512], in_=halves[t],
                        func=AF.Identity, bias=bp_sb[:, m : m + 1],
                    )

            if debug_taps:
                nc.sync.dma_start(dbg["d_pj"][:], proj[:])

            # ---------- GN2 + residual (via bf16 DMA accumulate) ----------
            gn_finish(stat22, g2s, g2b, ab2, "2", psg_tag="s")
            for k in range(KC):
                nc.vector.tensor_scalar(
                    stg[:, k, :], proj[:, k, :],
                    ab2[:, k, 0:1], ab2[:, k, 1:2], OP.mult, OP.add,
                )
                if k % 2 == 1:
                    nc.gpsimd.dma_start(
                        out_d[:, k - 1 : k + 1, :], stg[:, k - 1 : k + 1, :],
                        accum_op=OP.add,
                    )

    nc.compile()
    return nc


def _host_prep(x, gn1_scale, gn1_bias, w_qkv, b_qkv, w_proj, b_proj, gn2_scale, gn2_bias):
    """Build per-core input maps (numpy only)."""
    import ml_dtypes

    f = np.float32
    bf = ml_dtypes.bfloat16
    x = np.asarray(x, f)
    w_qkv = np.asarray(w_qkv, f)
    b_qkv = np.asarray(b_qkv, f)
    w_proj = np.asarray(w_proj, f)
    b_proj = np.asarray(b_proj, f)
    gn1_scale = np.asarray(gn1_scale, f)
    gn1_bias = np.asarray(gn1_bias, f)
    gn2_scale = np.asarray(gn2_scale, f)
    gn2_bias = np.asarray(gn2_bias, f)

    def chunk_vec(v):  # [C] -> [P, KC]
        return np.ascontiguousarray(v.reshape(KC, P).T)

    def chunk_mat(wt):  # [C, M] -> [P, KC, M]
        return np.ascontiguousarray(wt.reshape(KC, P, -1).transpose(1, 0, 2))

    def mtile(w):  # [P, KC, C] -> [P, M, KC, P]
        return np.ascontiguousarray(w.reshape(P, KC, KC, P).transpose(0, 2, 1, 3))

    idx = np.arange(NHEAD)[:, None] * (3 * CH) + np.arange(CH)[None, :]
    q_idx, k_idx, v_idx = idx.ravel(), (idx + CH).ravel(), (idx + 2 * CH).ravel()

    s4 = float(CH) ** -0.25
    wq = mtile(chunk_mat(w_qkv[q_idx].T * (s4 * ALPHA)))
    wk = mtile(chunk_mat(w_qkv[k_idx].T * (s4 * ALPHA)))
    wv = chunk_mat(w_qkv[v_idx].T)
    wp = chunk_mat(w_proj.T).astype(bf)
    biases = np.stack(
        [
            chunk_vec(b_qkv[q_idx] * (s4 * ALPHA)),
            chunk_vec(b_qkv[k_idx] * (s4 * ALPHA)),
            chunk_vec(b_qkv[v_idx]),
            chunk_vec(b_proj),
            chunk_vec(gn1_scale), chunk_vec(gn1_bias),
            chunk_vec(gn2_scale), chunk_vec(gn2_bias),
        ],
        axis=1,
    )

    cidx = np.arange(C)
    sel = np.zeros((P, KC, GROUPS), f)
    sel[cidx % P, cidx // P, cidx // 16] = 1.0 / 16.0
    selt = np.zeros((GROUPS, KC, P), f)
    selt[cidx // 16, cidx // P, cidx % P] = 1.0
    ident = np.eye(P, dtype=bf)

    shared = {
        "wq": wq, "wk": wk, "wv": wv, "wp": wp,
        "biases": np.ascontiguousarray(biases),
        "sel": sel, "selt": selt, "ident": ident,
    }
    in_maps = []
    for b in range(B):
        xb = np.ascontiguousarray(
            x[b].reshape(C, N).reshape(KC, P, N).transpose(1, 0, 2)
        ).astype(bf)
        in_maps.append({"x": xb, **shared})
    return in_maps


def _assemble(results):
    out = np.empty((B, C, H, W), np.float32)
    for b in range(B):
        ob = np.asarray(results[b]["out"]).astype(np.float32)  # [P, KC, N]
        out[b] = ob.transpose(1, 0, 2).reshape(C, N).reshape(C, H, W)
    return out


def get_nc():
    if "nc" not in _CACHE:
        _CACHE["nc"] = _build_nc()
    return _CACHE["nc"]


def kernel(x, gn1_scale, gn1_bias, w_qkv, b_qkv, w_proj, b_proj, gn2_scale, gn2_bias):
    from concourse.bass_utils import run_bass_kernel_spmd

    nc = get_nc()
    in_maps = _host_prep(
        x, gn1_scale, gn1_bias, w_qkv, b_qkv, w_proj, b_proj, gn2_scale, gn2_bias
    )
    res = run_bass_kernel_spmd(nc, in_maps, core_ids=list(range(B)))
    return _assemble(res.results)
